# revision 14
# baseline (speedup 1.0000x reference)
"""GCN (3-layer + mean-pool + FC) on 8 Trainium2 NeuronCores via Bass.

Self-contained: host-side numpy preprocessing shards nodes (at graph
boundaries) and edges (by destination) across 8 cores, builds one SPMD
Bass program, runs it via run_bass_kernel_spmd, and reassembles the
full [512, 11] output.

Algorithm per GCN layer (h' := dis * (g @ W), dis := rsqrt(deg+1)):
  agg[d] = sum_{e: dst=d} (ew_e * dis_d) * h'[src_e]  +  dis_d * h'[d] + b
computed as dense selection matmuls.  Both dis factors are folded into
host data: ew' = ew * dis_dst scales the edge-selection matrix Msel, and
the self-loop uses rhs = diag(dis) slices, so the epilogue is a single
Relu+bias activation.

Edges are sorted by destination, cut into 128-slot chunks whose
destinations fall inside 32-wide windows of a 512-dst block; per chunk
PSUM accumulates S_T[f, dw] += gathered[slot, f]^T @ Msel[slot, dw]
with Msel[slot, dw] = (iota == dstw[slot]) * ew'[slot] built in two DVE
ops per run, and the self-loop added via diag(dis) matmuls of the local
h' block (which also initialize the PSUM window).

The per-edge source-row gather uses the SWDGE dma_gather instruction:
one gather per (dst-block, table-quarter, <=8 chunks), where the
allgathered bf16 h' table (8*N_LOC rows) is addressed in 2*N_LOC-row
quarters so row indices fit in int16.  Chunks are quarter-pure; chunk
structure is uniform across cores (max over cores per cell) so one SPMD
program serves all 8.
"""
import sys
import numpy as np

sys.path.insert(0, '/opt/trn_rl_repo')

N_CORES = 8
F = 128          # feature dim (in = hid = 128)
OUT_DIM = 11
BLK = 512        # dst nodes per dst-block (psum + epilogue granularity)
WIN = 32         # dst window per chunk matmul
WPB = BLK // WIN
CPB = BLK // 128  # 128-col groups per block
WPC = 128 // WIN  # windows per 128-col group


def _preprocess(x, edge_index, batch, edge_weight, n_graphs):
    """Shard nodes/edges across cores; build per-core device arrays and the
    (core-uniform) chunk structure."""
    import ml_dtypes
    n_nodes = x.shape[0]
    batch = np.asarray(batch).astype(np.int64)
    src = np.asarray(edge_index[0]).astype(np.int64)
    dst = np.asarray(edge_index[1]).astype(np.int64)
    ew = np.asarray(edge_weight).astype(np.float32)

    # --- node shards cut at graph boundaries ---
    gstart = np.searchsorted(batch, np.arange(n_graphs + 1))
    cuts = [0]
    for c in range(1, N_CORES):
        target = c * n_nodes / N_CORES
        g = int(np.searchsorted(gstart, target))
        if g > 0 and target - gstart[g - 1] < gstart[g] - target:
            g = g - 1
        g = min(max(g, cuts[-1]), n_graphs)
        cuts.append(g)
    cuts.append(n_graphs)
    cuts = np.array(cuts)
    node_lo = gstart[cuts[:-1]]
    node_hi = gstart[cuts[1:]]
    n_loc_real = node_hi - node_lo
    N_LOC = int(np.ceil(max(n_loc_real.max(), 1) / BLK) * BLK)
    assert 2 * N_LOC <= 32767, "table quarter must fit int16 indices"
    NBLK = N_LOC // BLK
    NCOL = N_LOC // 128
    QS = 2 * N_LOC           # rows per table quarter
    n_graphs_core = cuts[1:] - cuts[:-1]
    G_LOC = int(n_graphs_core.max())

    owner = np.searchsorted(node_hi, np.arange(n_nodes), side='right')
    local = np.arange(n_nodes) - node_lo[owner]
    # quarter table: node (c, r) lives in quarter r//QN at row c*QN + r%QN
    QN = N_LOC // 4
    q_of = local // QN
    qrow = owner * QN + local % QN      # row within its quarter table

    # --- degree / dis on host (sym normalization with self-loop) ---
    deg_g = np.zeros(n_nodes, np.float64)
    np.add.at(deg_g, dst, ew.astype(np.float64))
    dis_g = (1.0 / np.sqrt(deg_g + 1.0)).astype(np.float32)

    # --- edges assigned by dst; cells = (block, src-quarter) ---
    e_core = owner[dst]
    e_dstl = local[dst]
    e_q = q_of[src]
    e_b = e_dstl // BLK
    NCELL = NBLK * 4
    e_key = e_b * 4 + e_q

    cnt = np.zeros((N_CORES, NCELL), np.int64)
    for c in range(N_CORES):
        np.add.at(cnt[c], e_key[e_core == c], 1)
    nch_cell = np.ceil(cnt.max(axis=0) / 128).astype(np.int64)  # [NCELL]
    cell_off = np.concatenate([[0], np.cumsum(nch_cell)])
    NCH = int(cell_off[-1])

    # program structure (python constants, identical for all cores).
    chunk_wins = [set() for _ in range(NCH)]
    blocks = []
    for b in range(NBLK):
        groups = []
        for q in range(4):
            cell = b * 4 + q
            c0 = int(cell_off[cell])
            nchg = int(cell_off[cell + 1] - c0)
            if nchg > 0:
                groups.append((q, c0, nchg))
        c0b = int(cell_off[b * 4])
        chb = int(cell_off[(b + 1) * 4]) - c0b
        blocks.append(dict(groups=groups, c0=c0b, chb=chb))
    MAXCHB = max(bl['chb'] for bl in blocks) if NCH else 1

    # --- per-core device arrays ---
    dstw = np.zeros((N_CORES, 128, max(NCH, 1)), np.float32)
    ewa = np.zeros((N_CORES, 128, max(NCH, 1)), np.float32)
    idx16 = np.zeros((N_CORES, 128, max(NCH, 1) * 8), np.int16)

    for c in range(N_CORES):
        m = np.where(e_core == c)[0]
        k = e_key[m]
        order = np.lexsort((e_dstl[m], k))   # cell-major, dst minor
        me = m[order]
        ks = k[order]
        cell_start = np.searchsorted(ks, np.arange(NCELL))
        pos = np.arange(len(me)) - cell_start[ks]
        slot = cell_off[ks] * 128 + pos
        cid = slot // 128
        p = slot % 128
        dstw[c, p, cid] = (e_dstl[me] % BLK).astype(np.float32) - 256.0
        ewa[c, p, cid] = ew[me] * dis_g[dst[me]]
        idxv = qrow[src[me]].astype(np.int16)
        col = cid * 8 + p // 16
        for r in range(8):
            idx16[c, 16 * r + p % 16, col] = idxv
        for cw in set(zip(cid.tolist(), ((e_dstl[me] % BLK) // WIN).tolist())):
            chunk_wins[cw[0]].add(cw[1])

    # --- emission positions (window-major per block) + per-emission arrays:
    # dstwE[slot, pos] = dst_local%BLK - j*WIN for emission (window j, chunk
    # cid); matches iotaW = 0..WIN-1 exactly in bf16 (mismatches stay outside
    # [0, WIN)).  ewE is the ew' value replicated per emission of the chunk.
    pos_of_b = []       # per block: {(j, cid): pos}
    e0_b = []           # per block: base emission position
    NEBT = 0
    for b in range(NBLK):
        bl = blocks[b]
        win_emits = [[] for _ in range(WPB)]
        for cid in range(bl['c0'], bl['c0'] + bl['chb']):
            for w in chunk_wins[cid]:
                win_emits[w].append(cid)
        pos_of = {}
        pos = 0
        for j in range(WPB):
            for cid in win_emits[j]:
                pos_of[(j, cid)] = pos
                pos += 1
        pos_of_b.append(pos_of)
        e0_b.append(NEBT)
        NEBT += pos
    MAXEB = max((len(p) for p in pos_of_b), default=1) or 1
    dstwE = np.full((N_CORES, 128, max(NEBT, 1)), -1.0, np.float32)
    ewE = np.zeros((N_CORES, 128, max(NEBT, 1)), np.float32)
    for b in range(NBLK):
        for (j, cid), pos in pos_of_b[b].items():
            gp = e0_b[b] + pos
            dstwE[:, :, gp] = dstw[:, :, cid] + (256.0 - j * WIN)
            ewE[:, :, gp] = ewa[:, :, cid]
    # padded slots (ew==0) get dstwE=-1 so they never match
    dstwE[ewE == 0.0] = -1.0

    gid = np.full((N_CORES, 128, NCOL), -1.0, np.float32)
    invn = np.ones((N_CORES, 128, NCOL), np.float32)
    dis_a = np.zeros((N_CORES, 128, NCOL), np.float32)
    eyedis = np.zeros((N_CORES, 128, NCOL * 128), ml_dtypes.bfloat16)
    gcnt = np.bincount(batch, minlength=n_graphs).astype(np.float32)
    ar = np.arange(128)
    for c in range(N_CORES):
        n = n_loc_real[c]
        ids = np.arange(node_lo[c], node_hi[c])
        rel = batch[ids] - cuts[c]
        li = np.arange(n)
        gid[c, li % 128, li // 128] = rel.astype(np.float32)
        invn[c, li % 128, li // 128] = 1.0 / gcnt[batch[ids]]
        dis_a[c, li % 128, li // 128] = dis_g[ids]
        for col in range(NCOL):
            eyedis[c, ar, col * 128 + ar] = dis_a[c, :, col].astype(
                ml_dtypes.bfloat16)

    xT = np.zeros((N_CORES, 128, N_LOC), ml_dtypes.bfloat16)
    for c in range(N_CORES):
        n = n_loc_real[c]
        xT[c, :, :n] = np.asarray(x[node_lo[c]:node_hi[c]]).astype(np.float32).T

    meta = dict(N_LOC=N_LOC, NBLK=NBLK, NCOL=NCOL, NCH=max(NCH, 1), QS=QS,
                blocks=blocks, MAXCHB=MAXCHB, G_LOC=G_LOC, MAXEB=MAXEB,
                NEBT=max(NEBT, 1), pos_of_b=pos_of_b, e0_b=e0_b,
                chunk_wins=[sorted(s) for s in chunk_wins],
                n_graphs_core=n_graphs_core.tolist())
    arrays = dict(xT=xT, dstwE=dstwE, ewE=ewE, idx16=idx16,
                  gid=gid, invn=invn, dis=dis_a, eyedis=eyedis)
    return meta, arrays


def _build_program(meta):
    from concourse import bass, bacc, tile, mybir

    N_LOC, NBLK, NCH = meta['N_LOC'], meta['NBLK'], meta['NCH']
    NCOL, QS = meta['NCOL'], meta['QS']
    blocks, MAXCHB = meta['blocks'], meta['MAXCHB']
    G_LOC, MAXEB, NEBT = meta['G_LOC'], meta['MAXEB'], meta['NEBT']

    nc = bacc.Bacc("TRN2", target_bir_lowering=False, debug=False,
                   num_devices=N_CORES, num_swdge_queues=4)
    f32, bf16, i16 = mybir.dt.float32, mybir.dt.bfloat16, mybir.dt.int16
    AF = mybir.ActivationFunctionType
    OP = mybir.AluOpType

    xT_in = nc.dram_tensor("xT", [128, N_LOC], bf16, kind="ExternalInput")
    dstw_in = nc.dram_tensor("dstwE", [128, NEBT], bf16, kind="ExternalInput")
    ew_in = nc.dram_tensor("ewE", [128, NEBT], bf16, kind="ExternalInput")
    idx_in = nc.dram_tensor("idx16", [128, NCH * 8], i16, kind="ExternalInput")
    gid_in = nc.dram_tensor("gid", [128, NCOL], f32, kind="ExternalInput")
    invn_in = nc.dram_tensor("invn", [128, NCOL], f32, kind="ExternalInput")
    dis_in = nc.dram_tensor("dis", [128, NCOL], f32, kind="ExternalInput")
    eyedis_in = nc.dram_tensor("eyedis", [128, NCOL * 128], bf16,
                               kind="ExternalInput")
    iota_in = nc.dram_tensor("iotaW", [128, WIN], bf16, kind="ExternalInput")
    iotaG_in = nc.dram_tensor("iotaG", [128, G_LOC], f32, kind="ExternalInput")
    eye_in = nc.dram_tensor("eye", [128, 128], bf16, kind="ExternalInput")
    W_in = [nc.dram_tensor(f"W{l}", [128, 128], bf16, kind="ExternalInput") for l in (1, 2, 3)]
    b12_in = [nc.dram_tensor(f"b{l}", [128, 1], f32, kind="ExternalInput") for l in (1, 2, 3)]
    fcw_in = nc.dram_tensor("fcw", [128, OUT_DIM], f32, kind="ExternalInput")
    fcb_in = nc.dram_tensor("fcbrep", [128, OUT_DIM], f32, kind="ExternalInput")
    y_out = nc.dram_tensor("y", [G_LOC, OUT_DIM], f32, kind="ExternalOutput")

    with tile.TileContext(nc) as tc:
        with (
            tc.tile_pool(name="const", bufs=1) as cpool,
            tc.tile_pool(name="big", bufs=1) as bigpool,
            tc.tile_pool(name="gat", bufs=3) as gatpool,
            tc.tile_pool(name="msel", bufs=3) as mselpool,
            tc.tile_pool(name="work", bufs=2) as workpool,
            tc.tile_pool(name="hcol", bufs=4) as hcolpool,
            tc.tile_pool(name="slst", bufs=3) as slpool,
            tc.tile_pool(name="hp", bufs=2, space="PSUM") as hpsum,
            tc.tile_pool(name="sp", bufs=2, space="PSUM") as spsum,
            tc.tile_pool(name="pp", bufs=1, space="PSUM") as ppsum,
            tc.tile_pool(name="dram", bufs=1, space="DRAM") as dpool,
        ):
            def load(shape, src, tag, dt=f32, pool=cpool):
                t = pool.tile(shape, dt, tag=tag)
                nc.sync.dma_start(t[:], src[:])
                return t
            dstw_t = load([128, NEBT], dstw_in, "dstwE", bf16)
            ew_t = load([128, NEBT], ew_in, "ewE", bf16)
            idx_t = load([128, NCH * 8], idx_in, "idx16", i16)
            gid_t = load([128, NCOL], gid_in, "gid")
            invn_t = load([128, NCOL], invn_in, "invn")
            dis_t = load([128, NCOL], dis_in, "dis")
            iota_t = load([128, WIN], iota_in, "iotaW", bf16)
            iotaG_t = load([128, G_LOC], iotaG_in, "iotaG")
            eye_bf = load([128, 128], eye_in, "eye", bf16)
            W_t = [load([128, 128], w, f"W{i}", bf16) for i, w in enumerate(W_in)]
            b12_t = [load([128, 1], b, f"b{i}") for i, b in enumerate(b12_in)]
            fcw_t = load([128, OUT_DIM], fcw_in, "fcw")
            fcb_t = load([128, OUT_DIM], fcb_in, "fcb")

            gbuf = bigpool.tile([128, N_LOC], bf16, tag="gbuf")
            QN = N_LOC // 4              # nodes per quarter table slice
            QCOL = QN // 128
            for j in range(4):           # quarter-granular load: phase A can
                nc.sync.dma_start(       # start before the whole x arrives
                    gbuf[:, j * QN:(j + 1) * QN],
                    xT_in[:, j * QN:(j + 1) * QN])

            # ---- layers (phase A of layer l+1 interleaved into layer l) ----
            qrr = [0]                    # gather queue round-robin counter
            ltabQ_l = [[dpool.tile([QN, 128], bf16, tag=f"ltab{li}q{j}",
                                   name=f"ltab{li}q{j}") for j in range(4)]
                       for li in range(3)]
            tableQ_l = [[dpool.tile([N_CORES * QN, 128], bf16,
                                    tag=f"table{li}q{j}", name=f"table{li}q{j}",
                                    addr_space="Shared")
                         for j in range(4)] for li in range(3)]

            def emit_phaseA_col(li, i):
                hp = hpsum.tile([128, 128], f32, tag="hp")
                nc.tensor.matmul(hp[:], lhsT=gbuf[:, i * 128:(i + 1) * 128],
                                 rhs=W_t[li][:], start=True, stop=True)
                hcol = hcolpool.tile([128, 128], bf16, tag="hcol")
                nc.scalar.activation(hcol[:], hp[:], AF.Copy,
                                     scale=dis_t[:, i:i + 1])
                j, ji = i // QCOL, i % QCOL
                nc.sync.dma_start(ltabQ_l[li][j][ji * 128:(ji + 1) * 128, :],
                                  hcol[:])
                if (i + 1) % QCOL == 0:
                    nc.gpsimd.collective_compute(
                        "AllGather", OP.bypass,
                        replica_groups=[list(range(N_CORES))],
                        ins=[ltabQ_l[li][j].opt()],
                        outs=[tableQ_l[li][j].opt()],
                    )

            for i in range(NCOL):
                emit_phaseA_col(0, i)

            for li in range(3):
                tableQ = tableQ_l[li]

                # epilogue emitted one block late so it never heads the
                # Scalar queue before the next block's work is issued
                pending_epi = []

                def flush_epi():
                    for fn in pending_epi:
                        fn()
                    pending_epi.clear()

                ncols_next = [0]

                def emit_next_phase(nblocks_done, li=li):
                    if li == 2:
                        return
                    target = min(nblocks_done * CPB, NCOL)
                    while ncols_next[0] < target:
                        emit_phaseA_col(li + 1, ncols_next[0])
                        ncols_next[0] += 1

                for b in range(NBLK):
                    bl = blocks[b]
                    gat = gatpool.tile([128, MAXCHB * 128], bf16, tag="gat")
                    for (q, c0, nchg) in bl['groups']:
                        for s0 in range(0, nchg, 8):   # <=1024 idx per gather
                            sn = min(8, nchg - s0)
                            c = c0 + s0
                            rel = c - bl['c0']
                            out_ap = gat[:, rel * 128:(rel + sn) * 128].rearrange(
                                "p (c f) -> p c f", f=128)
                            # first blocks of a layer: pin queue=quarter so
                            # queues 0..2 don't stall behind the last quarter's
                            # pending AllGather
                            qn = q if b < 3 else qrr[0] % 4
                            nc.gpsimd.dma_gather(
                                out_ap, tableQ[q][:],
                                idx_t[:, c * 8:(c + sn) * 8],
                                sn * 128, sn * 128, 128,
                                queue_num=qn)
                            qrr[0] += 1

                    sp = spsum.tile([128, BLK], f32, tag="sp")
                    # window-contiguous emission: PSUM accumulation groups
                    # must not interleave regions within a bank
                    win_emits = [[] for _ in range(WPB)]
                    for cid in range(bl['c0'], bl['c0'] + bl['chb']):
                        for w in meta['chunk_wins'][cid]:
                            win_emits[w].append(cid)
                    pos_of = meta['pos_of_b'][b]
                    e0 = meta['e0_b'][b]
                    neb = len(pos_of)
                    msall = mselpool.tile([128, MAXEB * WIN], bf16, tag="msall")
                    out3 = msall[:, :neb * WIN].rearrange(
                        "p (c f) -> p c f", f=WIN)
                    nc.vector.tensor_tensor(
                        out=out3,
                        in0=dstw_t[:, e0:e0 + neb].unsqueeze(2)
                            .broadcast_to([128, neb, WIN]),
                        in1=iota_t[:].unsqueeze(1)
                            .broadcast_to([128, neb, WIN]),
                        op=OP.is_equal)
                    nc.vector.tensor_tensor(
                        out=out3, in0=out3,
                        in1=ew_t[:, e0:e0 + neb].unsqueeze(2)
                            .broadcast_to([128, neb, WIN]),
                        op=OP.mult)
                    slst = slpool.tile([128, CPB * 128], bf16, tag="slst")
                    for jj in range(CPB):
                        col = b * CPB + jj
                        qj, ji = col // QCOL, col % QCOL
                        nc.sync.dma_start(
                            slst[:, jj * 128:(jj + 1) * 128],
                            ltabQ_l[li][qj][ji * 128:(ji + 1) * 128, :])
                    edis = slpool.tile([128, CPB * 128], bf16, tag="edis")
                    nc.sync.dma_start(
                        edis[:], eyedis_in[:, b * CPB * 128:(b + 1) * CPB * 128])
                    for j in range(WPB):
                        hblk = slst[:, (j // WPC) * 128:(j // WPC + 1) * 128]
                        wc = win_emits[j]
                        nc.tensor.matmul(
                            sp[:, j * WIN:(j + 1) * WIN],
                            lhsT=hblk,
                            rhs=edis[:, (j // WPC) * 128 + (j % WPC) * WIN:
                                     (j // WPC) * 128 + (j % WPC + 1) * WIN],
                            start=True, stop=(len(wc) == 0),
                            skip_group_check=True)
                        for n, cid in enumerate(wc):
                            p0 = pos_of[(j, cid)]
                            r = cid - bl['c0']
                            nc.tensor.matmul(sp[:, j * WIN:(j + 1) * WIN],
                                             lhsT=gat[:, r * 128:(r + 1) * 128],
                                             rhs=msall[:, p0 * WIN:(p0 + 1) * WIN],
                                             start=False, stop=(n == len(wc) - 1),
                                             skip_group_check=True)

                    def make_epi(b=b, sp=sp, li=li):
                        gslice = gbuf[:, b * BLK:(b + 1) * BLK]
                        nc.scalar.activation(gslice, sp[:], AF.Relu,
                                             bias=b12_t[li][:])
                    flush_epi()
                    emit_next_phase(b)
                    pending_epi.append(make_epi)
                flush_epi()
                emit_next_phase(NBLK)

            # ---- pooling (gbuf holds g3 node-feature-major; transpose per column) ----
            pp = ppsum.tile([128, G_LOC], f32, tag="pp")
            for i in range(NCOL):
                tp = hpsum.tile([128, 128], bf16, tag="hptp")
                nc.tensor.transpose(tp[:], gbuf[:, i * 128:(i + 1) * 128],
                                    eye_bf[:])
                g3n = workpool.tile([128, 128], bf16, tag="g3n")
                nc.scalar.activation(g3n[:], tp[:], AF.Copy)
                P = mselpool.tile([128, G_LOC], bf16, tag="P")
                nc.vector.tensor_scalar(
                    out=P[:], in0=iotaG_t[:], scalar1=gid_t[:, i:i + 1],
                    scalar2=invn_t[:, i:i + 1], op0=OP.is_equal, op1=OP.mult)
                nc.tensor.matmul(pp[:], lhsT=g3n[:], rhs=P[:],
                                 start=(i == 0), stop=(i == NCOL - 1),
                                 skip_group_check=True)
            pooledT = cpool.tile([128, G_LOC], f32, tag="pooledT")
            nc.vector.tensor_copy(pooledT[:], pp[:])

            fp = ppsum.tile([128, OUT_DIM], f32, tag="fc")
            nc.tensor.matmul(fp[:G_LOC, :], lhsT=pooledT[:], rhs=fcw_t[:],
                             start=True, stop=True)
            yt = cpool.tile([128, OUT_DIM], f32, tag="yt")
            nc.vector.tensor_tensor(out=yt[:G_LOC, :], in0=fp[:G_LOC, :],
                                    in1=fcb_t[:G_LOC, :], op=OP.add)
            nc.sync.dma_start(y_out[:], yt[:G_LOC, :])

    nc.compile()
    return nc


def _make_in_maps(meta, arrays, W1, b1, W2, b2, W3, b3, fcW, fcb):
    import ml_dtypes
    G_LOC = meta['G_LOC']
    iotaW = np.broadcast_to(np.arange(WIN, dtype=np.float32),
                            (128, WIN)).astype(ml_dtypes.bfloat16).copy()
    iotaG = np.broadcast_to(np.arange(G_LOC, dtype=np.float32), (128, G_LOC)).copy()
    eye = np.eye(128, dtype=np.float32).astype(ml_dtypes.bfloat16)
    fcbrep = np.broadcast_to(np.asarray(fcb, np.float32), (128, OUT_DIM)).copy()
    common = {
        "iotaW": iotaW, "iotaG": iotaG, "eye": eye,
        "W1": np.asarray(W1, np.float32).astype(ml_dtypes.bfloat16),
        "W2": np.asarray(W2, np.float32).astype(ml_dtypes.bfloat16),
        "W3": np.asarray(W3, np.float32).astype(ml_dtypes.bfloat16),
        "b1": np.asarray(b1, np.float32).reshape(128, 1),
        "b2": np.asarray(b2, np.float32).reshape(128, 1),
        "b3": np.asarray(b3, np.float32).reshape(128, 1),
        "fcw": np.asarray(fcW, np.float32),
        "fcbrep": fcbrep,
    }
    in_maps = []
    for c in range(N_CORES):
        m = dict(common)
        for k in ("xT", "idx16", "gid", "invn", "dis", "eyedis"):
            m[k] = arrays[k][c]
        m["dstwE"] = arrays["dstwE"][c].astype(ml_dtypes.bfloat16)
        m["ewE"] = arrays["ewE"][c].astype(ml_dtypes.bfloat16)
        in_maps.append(m)
    return in_maps


def run(x, edge_index, batch, edge_weight, W1, b1, W2, b2, W3, b3, fcW, fcb,
        n_graphs=512, trace=False):
    from concourse import bass_utils
    meta, arrays = _preprocess(x, edge_index, batch, edge_weight, n_graphs)
    nc = _build_program(meta)
    in_maps = _make_in_maps(meta, arrays, W1, b1, W2, b2, W3, b3, fcW, fcb)
    res = bass_utils.run_bass_kernel_spmd(
        nc, in_maps, core_ids=list(range(N_CORES)), trace=trace)
    ng = meta['n_graphs_core']
    y = np.concatenate([res.results[c]["y"][:ng[c]] for c in range(N_CORES)], axis=0)
    return y.astype(np.float32), res


def kernel(x, edge_index, batch, edge_weight, W1, b1, W2, b2, W3, b3, fcW, fcb):
    y, _ = run(np.asarray(x), np.asarray(edge_index), np.asarray(batch),
               np.asarray(edge_weight), W1, b1, W2, b2, W3, b3, fcW, fcb,
               n_graphs=512, trace=False)
    return y


# revision 15
# speedup vs baseline: 1.1927x; 1.1927x over previous
"""GCN (3-layer + mean-pool + FC) on 8 Trainium2 NeuronCores via Bass.

Self-contained: host-side numpy preprocessing shards nodes (at graph
boundaries) and edges (by destination) across 8 cores, builds one SPMD
Bass program, runs it via run_bass_kernel_spmd, and reassembles the
full [512, 11] output.

Algorithm per GCN layer (h' := dis * (g @ W), dis := rsqrt(deg+1)):
  agg[d] = sum_{e: dst=d} (ew_e * dis_d) * h'[src_e]  +  dis_d * h'[d] + b
computed as dense selection matmuls.  Both dis factors are folded into
host data: ew' = ew * dis_dst scales the edge-selection matrix Msel, and
the self-loop uses rhs = diag(dis) slices, so the epilogue is a single
Relu+bias activation.

Edges are sorted by destination, cut into 128-slot chunks whose
destinations fall inside 32-wide windows of a 512-dst block; per chunk
PSUM accumulates S_T[f, dw] += gathered[slot, f]^T @ Msel[slot, dw]
with Msel[slot, dw] = (iota == dstw[slot]) * ew'[slot] built in two DVE
ops per run, and the self-loop added via diag(dis) matmuls of the local
h' block (which also initialize the PSUM window).

The per-edge source-row gather uses the SWDGE dma_gather instruction:
one gather per (dst-block, table-quarter, <=8 chunks), where the
allgathered bf16 h' table (8*N_LOC rows) is addressed in 2*N_LOC-row
quarters so row indices fit in int16.  Chunks are quarter-pure; chunk
structure is uniform across cores (max over cores per cell) so one SPMD
program serves all 8.
"""
import sys
import numpy as np

sys.path.insert(0, '/opt/trn_rl_repo')

N_CORES = 8
F = 128          # feature dim (in = hid = 128)
OUT_DIM = 11
BLK = 512        # dst nodes per dst-block (psum + epilogue granularity)
WIN = 32         # dst window per chunk matmul
WPB = BLK // WIN
CPB = BLK // 128  # 128-col groups per block
WPC = 128 // WIN  # windows per 128-col group


def _preprocess(x, edge_index, batch, edge_weight, n_graphs):
    """Shard nodes/edges across cores; build per-core device arrays and the
    (core-uniform) chunk structure."""
    import ml_dtypes
    n_nodes = x.shape[0]
    batch = np.asarray(batch).astype(np.int64)
    src = np.asarray(edge_index[0]).astype(np.int64)
    dst = np.asarray(edge_index[1]).astype(np.int64)
    ew = np.asarray(edge_weight).astype(np.float32)

    # --- node shards cut at graph boundaries ---
    gstart = np.searchsorted(batch, np.arange(n_graphs + 1))
    cuts = [0]
    for c in range(1, N_CORES):
        target = c * n_nodes / N_CORES
        g = int(np.searchsorted(gstart, target))
        if g > 0 and target - gstart[g - 1] < gstart[g] - target:
            g = g - 1
        g = min(max(g, cuts[-1]), n_graphs)
        cuts.append(g)
    cuts.append(n_graphs)
    cuts = np.array(cuts)
    node_lo = gstart[cuts[:-1]]
    node_hi = gstart[cuts[1:]]
    n_loc_real = node_hi - node_lo
    N_LOC = int(np.ceil(max(n_loc_real.max(), 1) / BLK) * BLK)
    assert 2 * N_LOC <= 32767, "table quarter must fit int16 indices"
    NBLK = N_LOC // BLK
    NCOL = N_LOC // 128
    QS = 2 * N_LOC           # rows per table quarter
    n_graphs_core = cuts[1:] - cuts[:-1]
    G_LOC = int(n_graphs_core.max())

    owner = np.searchsorted(node_hi, np.arange(n_nodes), side='right')
    local = np.arange(n_nodes) - node_lo[owner]
    # quarter table: node (c, r) lives in quarter r//QN at row c*QN + r%QN
    QN = N_LOC // 4
    q_of = local // QN
    qrow = owner * QN + local % QN      # row within its quarter table

    # --- degree / dis on host (sym normalization with self-loop) ---
    deg_g = np.zeros(n_nodes, np.float64)
    np.add.at(deg_g, dst, ew.astype(np.float64))
    dis_g = (1.0 / np.sqrt(deg_g + 1.0)).astype(np.float32)

    # --- edges assigned by dst; cells = (block, src-quarter) ---
    e_core = owner[dst]
    e_dstl = local[dst]
    e_q = q_of[src]
    e_b = e_dstl // BLK
    NCELL = NBLK * 4
    e_key = e_b * 4 + e_q

    cnt = np.zeros((N_CORES, NCELL), np.int64)
    for c in range(N_CORES):
        np.add.at(cnt[c], e_key[e_core == c], 1)
    nch_cell = np.ceil(cnt.max(axis=0) / 128).astype(np.int64)  # [NCELL]
    cell_off = np.concatenate([[0], np.cumsum(nch_cell)])
    NCH = int(cell_off[-1])

    # program structure (python constants, identical for all cores).
    chunk_wins = [set() for _ in range(NCH)]
    blocks = []
    for b in range(NBLK):
        groups = []
        for q in range(4):
            cell = b * 4 + q
            c0 = int(cell_off[cell])
            nchg = int(cell_off[cell + 1] - c0)
            if nchg > 0:
                groups.append((q, c0, nchg))
        c0b = int(cell_off[b * 4])
        chb = int(cell_off[(b + 1) * 4]) - c0b
        blocks.append(dict(groups=groups, c0=c0b, chb=chb))
    MAXCHB = max(bl['chb'] for bl in blocks) if NCH else 1

    # --- per-core device arrays ---
    dstw = np.zeros((N_CORES, 128, max(NCH, 1)), np.float32)
    ewa = np.zeros((N_CORES, 128, max(NCH, 1)), np.float32)
    idx16 = np.zeros((N_CORES, 128, max(NCH, 1) * 8), np.int16)

    for c in range(N_CORES):
        m = np.where(e_core == c)[0]
        k = e_key[m]
        order = np.lexsort((e_dstl[m], k))   # cell-major, dst minor
        me = m[order]
        ks = k[order]
        cell_start = np.searchsorted(ks, np.arange(NCELL))
        pos = np.arange(len(me)) - cell_start[ks]
        slot = cell_off[ks] * 128 + pos
        cid = slot // 128
        p = slot % 128
        dstw[c, p, cid] = (e_dstl[me] % BLK).astype(np.float32) - 256.0
        ewa[c, p, cid] = ew[me] * dis_g[dst[me]]
        idxv = qrow[src[me]].astype(np.int16)
        col = cid * 8 + p // 16
        for r in range(8):
            idx16[c, 16 * r + p % 16, col] = idxv
        for cw in set(zip(cid.tolist(), ((e_dstl[me] % BLK) // WIN).tolist())):
            chunk_wins[cw[0]].add(cw[1])

    # --- emission positions (window-major per block) + per-emission arrays:
    # dstwE[slot, pos] = dst_local%BLK - j*WIN for emission (window j, chunk
    # cid); matches iotaW = 0..WIN-1 exactly in bf16 (mismatches stay outside
    # [0, WIN)).  ewE is the ew' value replicated per emission of the chunk.
    pos_of_b = []       # per block: {(j, cid): pos}
    e0_b = []           # per block: base emission position
    NEBT = 0
    for b in range(NBLK):
        bl = blocks[b]
        win_emits = [[] for _ in range(WPB)]
        for cid in range(bl['c0'], bl['c0'] + bl['chb']):
            for w in chunk_wins[cid]:
                win_emits[w].append(cid)
        pos_of = {}
        pos = 0
        for j in range(WPB):
            for cid in win_emits[j]:
                pos_of[(j, cid)] = pos
                pos += 1
        pos_of_b.append(pos_of)
        e0_b.append(NEBT)
        NEBT += pos
    MAXEB = max((len(p) for p in pos_of_b), default=1) or 1
    dstwE = np.full((N_CORES, 128, max(NEBT, 1)), -1.0, np.float32)
    ewE = np.zeros((N_CORES, 128, max(NEBT, 1)), np.float32)
    for b in range(NBLK):
        for (j, cid), pos in pos_of_b[b].items():
            gp = e0_b[b] + pos
            dstwE[:, :, gp] = dstw[:, :, cid] + (256.0 - j * WIN)
            ewE[:, :, gp] = ewa[:, :, cid]
    # padded slots (ew==0) get dstwE=-1 so they never match
    dstwE[ewE == 0.0] = -1.0

    gid = np.full((N_CORES, 128, NCOL), -1.0, np.float32)
    invn = np.ones((N_CORES, 128, NCOL), np.float32)
    dis_a = np.zeros((N_CORES, 128, NCOL), np.float32)
    eyedis = np.zeros((N_CORES, 128, NCOL * 128), ml_dtypes.bfloat16)
    gcnt = np.bincount(batch, minlength=n_graphs).astype(np.float32)
    ar = np.arange(128)
    for c in range(N_CORES):
        n = n_loc_real[c]
        ids = np.arange(node_lo[c], node_hi[c])
        rel = batch[ids] - cuts[c]
        li = np.arange(n)
        gid[c, li % 128, li // 128] = rel.astype(np.float32)
        invn[c, li % 128, li // 128] = 1.0 / gcnt[batch[ids]]
        dis_a[c, li % 128, li // 128] = dis_g[ids]
        for col in range(NCOL):
            eyedis[c, ar, col * 128 + ar] = dis_a[c, :, col].astype(
                ml_dtypes.bfloat16)

    xT = np.zeros((N_CORES, 128, N_LOC), ml_dtypes.bfloat16)
    for c in range(N_CORES):
        n = n_loc_real[c]
        xT[c, :, :n] = np.asarray(x[node_lo[c]:node_hi[c]]).astype(np.float32).T

    meta = dict(N_LOC=N_LOC, NBLK=NBLK, NCOL=NCOL, NCH=max(NCH, 1), QS=QS,
                blocks=blocks, MAXCHB=MAXCHB, G_LOC=G_LOC, MAXEB=MAXEB,
                NEBT=max(NEBT, 1), pos_of_b=pos_of_b, e0_b=e0_b,
                chunk_wins=[sorted(s) for s in chunk_wins],
                n_graphs_core=n_graphs_core.tolist())
    arrays = dict(xT=xT, dstwE=dstwE, ewE=ewE, idx16=idx16,
                  gid=gid, invn=invn, dis=dis_a, eyedis=eyedis)
    return meta, arrays


def _build_program(meta):
    from concourse import bass, bacc, tile, mybir

    N_LOC, NBLK, NCH = meta['N_LOC'], meta['NBLK'], meta['NCH']
    NCOL, QS = meta['NCOL'], meta['QS']
    blocks, MAXCHB = meta['blocks'], meta['MAXCHB']
    G_LOC, MAXEB, NEBT = meta['G_LOC'], meta['MAXEB'], meta['NEBT']

    nc = bacc.Bacc("TRN2", target_bir_lowering=False, debug=False,
                   num_devices=N_CORES, num_swdge_queues=4)
    f32, bf16, i16 = mybir.dt.float32, mybir.dt.bfloat16, mybir.dt.int16
    AF = mybir.ActivationFunctionType
    OP = mybir.AluOpType

    xT_in = nc.dram_tensor("xT", [128, N_LOC], bf16, kind="ExternalInput")
    dstw_in = nc.dram_tensor("dstwE", [128, NEBT], bf16, kind="ExternalInput")
    ew_in = nc.dram_tensor("ewE", [128, NEBT], bf16, kind="ExternalInput")
    idx_in = nc.dram_tensor("idx16", [128, NCH * 8], i16, kind="ExternalInput")
    gid_in = nc.dram_tensor("gid", [128, NCOL], f32, kind="ExternalInput")
    invn_in = nc.dram_tensor("invn", [128, NCOL], f32, kind="ExternalInput")
    dis_in = nc.dram_tensor("dis", [128, NCOL], f32, kind="ExternalInput")
    eyedis_in = nc.dram_tensor("eyedis", [128, NCOL * 128], bf16,
                               kind="ExternalInput")
    iota_in = nc.dram_tensor("iotaW", [128, WIN], bf16, kind="ExternalInput")
    iotaG_in = nc.dram_tensor("iotaG", [128, G_LOC], f32, kind="ExternalInput")
    eye_in = nc.dram_tensor("eye", [128, 128], bf16, kind="ExternalInput")
    W_in = [nc.dram_tensor(f"W{l}", [128, 128], bf16, kind="ExternalInput") for l in (1, 2, 3)]
    b12_in = [nc.dram_tensor(f"b{l}", [128, 1], f32, kind="ExternalInput") for l in (1, 2, 3)]
    fcw_in = nc.dram_tensor("fcw", [128, OUT_DIM], f32, kind="ExternalInput")
    fcb_in = nc.dram_tensor("fcbrep", [128, OUT_DIM], f32, kind="ExternalInput")
    y_out = nc.dram_tensor("y", [G_LOC, OUT_DIM], f32, kind="ExternalOutput")

    with tile.TileContext(nc) as tc:
        with (
            tc.tile_pool(name="const", bufs=1) as cpool,
            tc.tile_pool(name="big", bufs=1) as bigpool,
            tc.tile_pool(name="gat", bufs=3) as gatpool,
            tc.tile_pool(name="msel", bufs=3) as mselpool,
            tc.tile_pool(name="work", bufs=2) as workpool,
            tc.tile_pool(name="hcol", bufs=4) as hcolpool,
            tc.tile_pool(name="slst", bufs=3) as slpool,
            tc.tile_pool(name="hp", bufs=2, space="PSUM") as hpsum,
            tc.tile_pool(name="sp", bufs=2, space="PSUM") as spsum,
            tc.tile_pool(name="pp", bufs=1, space="PSUM") as ppsum,
            tc.tile_pool(name="dram", bufs=1, space="DRAM") as dpool,
        ):
            def load(shape, src, tag, dt=f32, pool=cpool):
                t = pool.tile(shape, dt, tag=tag)
                nc.sync.dma_start(t[:], src[:])
                return t
            dstw_t = load([128, NEBT], dstw_in, "dstwE", bf16)
            ew_t = load([128, NEBT], ew_in, "ewE", bf16)
            idx_t = load([128, NCH * 8], idx_in, "idx16", i16)
            gid_t = load([128, NCOL], gid_in, "gid")
            invn_t = load([128, NCOL], invn_in, "invn")
            dis_t = load([128, NCOL], dis_in, "dis")
            iota_t = load([128, WIN], iota_in, "iotaW", bf16)
            iotaG_t = load([128, G_LOC], iotaG_in, "iotaG")
            eye_bf = load([128, 128], eye_in, "eye", bf16)
            W_t = [load([128, 128], w, f"W{i}", bf16) for i, w in enumerate(W_in)]
            b12_t = [load([128, 1], b, f"b{i}") for i, b in enumerate(b12_in)]
            fcw_t = load([128, OUT_DIM], fcw_in, "fcw")
            fcb_t = load([128, OUT_DIM], fcb_in, "fcb")

            gbuf = bigpool.tile([128, N_LOC], bf16, tag="gbuf")
            QN = N_LOC // 4              # nodes per quarter table slice
            QCOL = QN // 128
            for j in range(4):           # quarter-granular load: phase A can
                nc.sync.dma_start(       # start before the whole x arrives
                    gbuf[:, j * QN:(j + 1) * QN],
                    xT_in[:, j * QN:(j + 1) * QN])

            # ---- layers (phase A of layer l+1 interleaved into layer l) ----
            qrr = [0]                    # gather queue round-robin counter
            ltabQ_l = [[dpool.tile([QN, 128], bf16, tag=f"ltab{li}q{j}",
                                   name=f"ltab{li}q{j}") for j in range(4)]
                       for li in range(3)]
            tableQ_l = [[dpool.tile([N_CORES * QN, 128], bf16,
                                    tag=f"table{li}q{j}", name=f"table{li}q{j}",
                                    addr_space="Shared")
                         for j in range(4)] for li in range(3)]

            def emit_phaseA_col(li, i):
                hp = hpsum.tile([128, 128], f32, tag="hp")
                nc.tensor.matmul(hp[:], lhsT=gbuf[:, i * 128:(i + 1) * 128],
                                 rhs=W_t[li][:], start=True, stop=True)
                hcol = hcolpool.tile([128, 128], bf16, tag="hcol")
                nc.scalar.activation(hcol[:], hp[:], AF.Copy,
                                     scale=dis_t[:, i:i + 1])
                j, ji = i // QCOL, i % QCOL
                nc.sync.dma_start(ltabQ_l[li][j][ji * 128:(ji + 1) * 128, :],
                                  hcol[:])
                if (i + 1) % QCOL == 0:
                    nc.gpsimd.collective_compute(
                        "AllGather", OP.bypass,
                        replica_groups=[list(range(N_CORES))],
                        ins=[ltabQ_l[li][j].opt()],
                        outs=[tableQ_l[li][j].opt()],
                    )

            for i in range(NCOL):
                emit_phaseA_col(0, i)

            for li in range(3):
                tableQ = tableQ_l[li]

                # epilogue emitted one block late so it never heads the
                # Scalar queue before the next block's work is issued
                pending_epi = []

                def flush_epi():
                    for fn in pending_epi:
                        fn()
                    pending_epi.clear()

                ncols_next = [0]

                def emit_next_phase(nblocks_done, li=li):
                    if li == 2:
                        return
                    target = min(nblocks_done * CPB, NCOL)
                    while ncols_next[0] < target:
                        emit_phaseA_col(li + 1, ncols_next[0])
                        ncols_next[0] += 1

                for b in range(NBLK):
                    bl = blocks[b]
                    gat = gatpool.tile([128, MAXCHB * 128], bf16, tag="gat")
                    for (q, c0, nchg) in bl['groups']:
                        for s0 in range(0, nchg, 8):   # <=1024 idx per gather
                            sn = min(8, nchg - s0)
                            c = c0 + s0
                            rel = c - bl['c0']
                            out_ap = gat[:, rel * 128:(rel + sn) * 128].rearrange(
                                "p (c f) -> p c f", f=128)
                            nc.gpsimd.dma_gather(
                                out_ap, tableQ[q][:],
                                idx_t[:, c * 8:(c + sn) * 8],
                                sn * 128, sn * 128, 128,
                                queue_num=qrr[0] % 4)
                            qrr[0] += 1

                    sp = spsum.tile([128, BLK], f32, tag="sp")
                    # window-contiguous emission: PSUM accumulation groups
                    # must not interleave regions within a bank
                    win_emits = [[] for _ in range(WPB)]
                    for cid in range(bl['c0'], bl['c0'] + bl['chb']):
                        for w in meta['chunk_wins'][cid]:
                            win_emits[w].append(cid)
                    pos_of = meta['pos_of_b'][b]
                    e0 = meta['e0_b'][b]
                    neb = len(pos_of)
                    msall = mselpool.tile([128, MAXEB * WIN], bf16, tag="msall")
                    out3 = msall[:, :neb * WIN].rearrange(
                        "p (c f) -> p c f", f=WIN)
                    nc.vector.tensor_tensor(
                        out=out3,
                        in0=dstw_t[:, e0:e0 + neb].unsqueeze(2)
                            .broadcast_to([128, neb, WIN]),
                        in1=iota_t[:].unsqueeze(1)
                            .broadcast_to([128, neb, WIN]),
                        op=OP.is_equal)
                    nc.vector.tensor_tensor(
                        out=out3, in0=out3,
                        in1=ew_t[:, e0:e0 + neb].unsqueeze(2)
                            .broadcast_to([128, neb, WIN]),
                        op=OP.mult)
                    slst = slpool.tile([128, CPB * 128], bf16, tag="slst")
                    for jj in range(CPB):
                        col = b * CPB + jj
                        qj, ji = col // QCOL, col % QCOL
                        nc.sync.dma_start(
                            slst[:, jj * 128:(jj + 1) * 128],
                            ltabQ_l[li][qj][ji * 128:(ji + 1) * 128, :])
                    edis = slpool.tile([128, CPB * 128], bf16, tag="edis")
                    nc.sync.dma_start(
                        edis[:], eyedis_in[:, b * CPB * 128:(b + 1) * CPB * 128])
                    for j in range(WPB):
                        hblk = slst[:, (j // WPC) * 128:(j // WPC + 1) * 128]
                        wc = win_emits[j]
                        nc.tensor.matmul(
                            sp[:, j * WIN:(j + 1) * WIN],
                            lhsT=hblk,
                            rhs=edis[:, (j // WPC) * 128 + (j % WPC) * WIN:
                                     (j // WPC) * 128 + (j % WPC + 1) * WIN],
                            start=True, stop=(len(wc) == 0),
                            skip_group_check=True)
                        for n, cid in enumerate(wc):
                            p0 = pos_of[(j, cid)]
                            r = cid - bl['c0']
                            nc.tensor.matmul(sp[:, j * WIN:(j + 1) * WIN],
                                             lhsT=gat[:, r * 128:(r + 1) * 128],
                                             rhs=msall[:, p0 * WIN:(p0 + 1) * WIN],
                                             start=False, stop=(n == len(wc) - 1),
                                             skip_group_check=True)

                    def make_epi(b=b, sp=sp, li=li):
                        gslice = gbuf[:, b * BLK:(b + 1) * BLK]
                        nc.scalar.activation(gslice, sp[:], AF.Relu,
                                             bias=b12_t[li][:])
                    flush_epi()
                    emit_next_phase(b)
                    pending_epi.append(make_epi)
                flush_epi()
                emit_next_phase(NBLK)

            # ---- pooling (gbuf holds g3 node-feature-major; transpose per column) ----
            pp = ppsum.tile([128, G_LOC], f32, tag="pp")
            for i in range(NCOL):
                tp = hpsum.tile([128, 128], bf16, tag="hptp")
                nc.tensor.transpose(tp[:], gbuf[:, i * 128:(i + 1) * 128],
                                    eye_bf[:])
                g3n = workpool.tile([128, 128], bf16, tag="g3n")
                nc.scalar.activation(g3n[:], tp[:], AF.Copy)
                P = mselpool.tile([128, G_LOC], bf16, tag="P")
                nc.vector.tensor_scalar(
                    out=P[:], in0=iotaG_t[:], scalar1=gid_t[:, i:i + 1],
                    scalar2=invn_t[:, i:i + 1], op0=OP.is_equal, op1=OP.mult)
                nc.tensor.matmul(pp[:], lhsT=g3n[:], rhs=P[:],
                                 start=(i == 0), stop=(i == NCOL - 1),
                                 skip_group_check=True)
            pooledT = cpool.tile([128, G_LOC], f32, tag="pooledT")
            nc.vector.tensor_copy(pooledT[:], pp[:])

            fp = ppsum.tile([128, OUT_DIM], f32, tag="fc")
            nc.tensor.matmul(fp[:G_LOC, :], lhsT=pooledT[:], rhs=fcw_t[:],
                             start=True, stop=True)
            yt = cpool.tile([128, OUT_DIM], f32, tag="yt")
            nc.vector.tensor_tensor(out=yt[:G_LOC, :], in0=fp[:G_LOC, :],
                                    in1=fcb_t[:G_LOC, :], op=OP.add)
            nc.sync.dma_start(y_out[:], yt[:G_LOC, :])

    nc.compile()
    return nc


def _make_in_maps(meta, arrays, W1, b1, W2, b2, W3, b3, fcW, fcb):
    import ml_dtypes
    G_LOC = meta['G_LOC']
    iotaW = np.broadcast_to(np.arange(WIN, dtype=np.float32),
                            (128, WIN)).astype(ml_dtypes.bfloat16).copy()
    iotaG = np.broadcast_to(np.arange(G_LOC, dtype=np.float32), (128, G_LOC)).copy()
    eye = np.eye(128, dtype=np.float32).astype(ml_dtypes.bfloat16)
    fcbrep = np.broadcast_to(np.asarray(fcb, np.float32), (128, OUT_DIM)).copy()
    common = {
        "iotaW": iotaW, "iotaG": iotaG, "eye": eye,
        "W1": np.asarray(W1, np.float32).astype(ml_dtypes.bfloat16),
        "W2": np.asarray(W2, np.float32).astype(ml_dtypes.bfloat16),
        "W3": np.asarray(W3, np.float32).astype(ml_dtypes.bfloat16),
        "b1": np.asarray(b1, np.float32).reshape(128, 1),
        "b2": np.asarray(b2, np.float32).reshape(128, 1),
        "b3": np.asarray(b3, np.float32).reshape(128, 1),
        "fcw": np.asarray(fcW, np.float32),
        "fcbrep": fcbrep,
    }
    in_maps = []
    for c in range(N_CORES):
        m = dict(common)
        for k in ("xT", "idx16", "gid", "invn", "dis", "eyedis"):
            m[k] = arrays[k][c]
        m["dstwE"] = arrays["dstwE"][c].astype(ml_dtypes.bfloat16)
        m["ewE"] = arrays["ewE"][c].astype(ml_dtypes.bfloat16)
        in_maps.append(m)
    return in_maps


def run(x, edge_index, batch, edge_weight, W1, b1, W2, b2, W3, b3, fcW, fcb,
        n_graphs=512, trace=False):
    from concourse import bass_utils
    meta, arrays = _preprocess(x, edge_index, batch, edge_weight, n_graphs)
    nc = _build_program(meta)
    in_maps = _make_in_maps(meta, arrays, W1, b1, W2, b2, W3, b3, fcW, fcb)
    res = bass_utils.run_bass_kernel_spmd(
        nc, in_maps, core_ids=list(range(N_CORES)), trace=trace)
    ng = meta['n_graphs_core']
    y = np.concatenate([res.results[c]["y"][:ng[c]] for c in range(N_CORES)], axis=0)
    return y.astype(np.float32), res


def kernel(x, edge_index, batch, edge_weight, W1, b1, W2, b2, W3, b3, fcW, fcb):
    y, _ = run(np.asarray(x), np.asarray(edge_index), np.asarray(batch),
               np.asarray(edge_weight), W1, b1, W2, b2, W3, b3, fcW, fcb,
               n_graphs=512, trace=False)
    return y


# revision 18
# speedup vs baseline: 1.3053x; 1.0944x over previous
"""GCN (3-layer + mean-pool + FC) on 8 Trainium2 NeuronCores via Bass.

Self-contained: host-side numpy preprocessing shards nodes (at graph
boundaries) and edges (by destination) across 8 cores, builds one SPMD
Bass program, runs it via run_bass_kernel_spmd, and reassembles the
full [512, 11] output.

Algorithm per GCN layer (h' := dis * (g @ W), dis := rsqrt(deg+1)):
  agg[d] = sum_{e: dst=d} (ew_e * dis_d) * h'[src_e]  +  dis_d * h'[d] + b
computed as dense selection matmuls.  Both dis factors are folded into
host data: ew' = ew * dis_dst scales the edge-selection matrix Msel, and
the self-loop uses rhs = diag(dis) slices, so the epilogue is a single
Relu+bias activation.

Edges are sorted by destination, cut into 128-slot chunks whose
destinations fall inside 32-wide windows of a 512-dst block; per chunk
PSUM accumulates S_T[f, dw] += gathered[slot, f]^T @ Msel[slot, dw]
with Msel[slot, dw] = (iota == dstw[slot]) * ew'[slot] built in two DVE
ops per run, and the self-loop added via diag(dis) matmuls of the local
h' block (which also initialize the PSUM window).

The per-edge source-row gather uses the SWDGE dma_gather instruction:
one gather per (dst-block, table-quarter, <=8 chunks), where the
allgathered bf16 h' table (8*N_LOC rows) is addressed in 2*N_LOC-row
quarters so row indices fit in int16.  Chunks are quarter-pure; chunk
structure is uniform across cores (max over cores per cell) so one SPMD
program serves all 8.
"""
import sys
import numpy as np

sys.path.insert(0, '/opt/trn_rl_repo')

N_CORES = 8
F = 128          # feature dim (in = hid = 128)
OUT_DIM = 11
BLK = 512        # dst nodes per dst-block (psum + epilogue granularity)
WIN = 32         # dst window per chunk matmul
WPB = BLK // WIN
CPB = BLK // 128  # 128-col groups per block
WPC = 128 // WIN  # windows per 128-col group


def _preprocess(x, edge_index, batch, edge_weight, n_graphs):
    """Shard nodes/edges across cores; build per-core device arrays and the
    (core-uniform) chunk structure."""
    import ml_dtypes
    n_nodes = x.shape[0]
    batch = np.asarray(batch).astype(np.int64)
    src = np.asarray(edge_index[0]).astype(np.int64)
    dst = np.asarray(edge_index[1]).astype(np.int64)
    ew = np.asarray(edge_weight).astype(np.float32)

    # --- node shards cut at graph boundaries ---
    gstart = np.searchsorted(batch, np.arange(n_graphs + 1))
    cuts = [0]
    for c in range(1, N_CORES):
        target = c * n_nodes / N_CORES
        g = int(np.searchsorted(gstart, target))
        if g > 0 and target - gstart[g - 1] < gstart[g] - target:
            g = g - 1
        g = min(max(g, cuts[-1]), n_graphs)
        cuts.append(g)
    cuts.append(n_graphs)
    cuts = np.array(cuts)
    node_lo = gstart[cuts[:-1]]
    node_hi = gstart[cuts[1:]]
    n_loc_real = node_hi - node_lo
    N_LOC = int(np.ceil(max(n_loc_real.max(), 1) / BLK) * BLK)
    assert 2 * N_LOC <= 32767, "table quarter must fit int16 indices"
    NBLK = N_LOC // BLK
    NCOL = N_LOC // 128
    QS = 2 * N_LOC           # rows per table quarter
    n_graphs_core = cuts[1:] - cuts[:-1]
    G_LOC = int(n_graphs_core.max())

    owner = np.searchsorted(node_hi, np.arange(n_nodes), side='right')
    local = np.arange(n_nodes) - node_lo[owner]
    # quarter table: node (c, r) lives in quarter r//QN at row c*QN + r%QN
    QN = N_LOC // 4
    q_of = local // QN
    qrow = owner * QN + local % QN      # row within its quarter table

    # --- degree / dis on host (sym normalization with self-loop) ---
    deg_g = np.zeros(n_nodes, np.float64)
    np.add.at(deg_g, dst, ew.astype(np.float64))
    dis_g = (1.0 / np.sqrt(deg_g + 1.0)).astype(np.float32)

    # --- edges assigned by dst; cells = (block, src-quarter) ---
    e_core = owner[dst]
    e_dstl = local[dst]
    e_q = q_of[src]
    e_b = e_dstl // BLK
    NCELL = NBLK * 4
    e_key = e_b * 4 + e_q

    cnt = np.zeros((N_CORES, NCELL), np.int64)
    for c in range(N_CORES):
        np.add.at(cnt[c], e_key[e_core == c], 1)
    nch_cell = np.ceil(cnt.max(axis=0) / 128).astype(np.int64)  # [NCELL]
    cell_off = np.concatenate([[0], np.cumsum(nch_cell)])
    NCH = int(cell_off[-1])

    # program structure (python constants, identical for all cores).
    chunk_wins = [set() for _ in range(NCH)]
    blocks = []
    for b in range(NBLK):
        groups = []
        for q in range(4):
            cell = b * 4 + q
            c0 = int(cell_off[cell])
            nchg = int(cell_off[cell + 1] - c0)
            if nchg > 0:
                groups.append((q, c0, nchg))
        c0b = int(cell_off[b * 4])
        chb = int(cell_off[(b + 1) * 4]) - c0b
        blocks.append(dict(groups=groups, c0=c0b, chb=chb))
    MAXCHB = max(bl['chb'] for bl in blocks) if NCH else 1

    # --- per-core device arrays ---
    dstw = np.zeros((N_CORES, 128, max(NCH, 1)), np.float32)
    ewa = np.zeros((N_CORES, 128, max(NCH, 1)), np.float32)
    idx16 = np.zeros((N_CORES, 128, max(NCH, 1) * 8), np.int16)

    for c in range(N_CORES):
        m = np.where(e_core == c)[0]
        k = e_key[m]
        order = np.lexsort((e_dstl[m], k))   # cell-major, dst minor
        me = m[order]
        ks = k[order]
        cell_start = np.searchsorted(ks, np.arange(NCELL))
        pos = np.arange(len(me)) - cell_start[ks]
        slot = cell_off[ks] * 128 + pos
        cid = slot // 128
        p = slot % 128
        dstw[c, p, cid] = (e_dstl[me] % BLK).astype(np.float32) - 256.0
        ewa[c, p, cid] = ew[me] * dis_g[dst[me]]
        idxv = qrow[src[me]].astype(np.int16)
        col = cid * 8 + p // 16
        for r in range(8):
            idx16[c, 16 * r + p % 16, col] = idxv
        for cw in set(zip(cid.tolist(), ((e_dstl[me] % BLK) // WIN).tolist())):
            chunk_wins[cw[0]].add(cw[1])

    # --- emission positions (window-major per block) + per-emission arrays:
    # dstwE[slot, pos] = dst_local%BLK - j*WIN for emission (window j, chunk
    # cid); matches iotaW = 0..WIN-1 exactly in bf16 (mismatches stay outside
    # [0, WIN)).  ewE is the ew' value replicated per emission of the chunk.
    pos_of_b = []       # per block: {(j, cid): pos}
    e0_b = []           # per block: base emission position
    wstart_b = []       # per block: position-start per window (len WPB+1)
    NEBT = 0
    for b in range(NBLK):
        bl = blocks[b]
        win_emits = [[] for _ in range(WPB)]
        for cid in range(bl['c0'], bl['c0'] + bl['chb']):
            for w in chunk_wins[cid]:
                win_emits[w].append(cid)
        pos_of = {}
        pos = 0
        wstart = []
        for j in range(WPB):
            wstart.append(pos)
            for cid in win_emits[j]:
                pos_of[(j, cid)] = pos
                pos += 1
        wstart.append(pos)
        pos_of_b.append(pos_of)
        wstart_b.append(wstart)
        e0_b.append(NEBT)
        NEBT += pos
    MAXEB = max((len(p) for p in pos_of_b), default=1) or 1
    dstwE = np.full((N_CORES, 128, max(NEBT, 1)), -1.0, np.float32)
    ewE = np.zeros((N_CORES, 128, max(NEBT, 1)), np.float32)
    for b in range(NBLK):
        for (j, cid), pos in pos_of_b[b].items():
            gp = e0_b[b] + pos
            dstwE[:, :, gp] = dstw[:, :, cid] + (256.0 - j * WIN)
            ewE[:, :, gp] = ewa[:, :, cid]
    # padded slots (ew==0) get dstwE=-1 so they never match
    dstwE[ewE == 0.0] = -1.0

    gid = np.full((N_CORES, 128, NCOL), -1.0, np.float32)
    invn = np.ones((N_CORES, 128, NCOL), np.float32)
    dis_a = np.zeros((N_CORES, 128, NCOL), np.float32)
    eyedis = np.zeros((N_CORES, 128, NCOL * 128), ml_dtypes.bfloat16)
    gcnt = np.bincount(batch, minlength=n_graphs).astype(np.float32)
    ar = np.arange(128)
    for c in range(N_CORES):
        n = n_loc_real[c]
        ids = np.arange(node_lo[c], node_hi[c])
        rel = batch[ids] - cuts[c]
        li = np.arange(n)
        gid[c, li % 128, li // 128] = rel.astype(np.float32)
        invn[c, li % 128, li // 128] = 1.0 / gcnt[batch[ids]]
        dis_a[c, li % 128, li // 128] = dis_g[ids]
        for col in range(NCOL):
            eyedis[c, ar, col * 128 + ar] = dis_a[c, :, col].astype(
                ml_dtypes.bfloat16)

    xT = np.zeros((N_CORES, 128, N_LOC), ml_dtypes.bfloat16)
    for c in range(N_CORES):
        n = n_loc_real[c]
        xT[c, :, :n] = np.asarray(x[node_lo[c]:node_hi[c]]).astype(np.float32).T

    meta = dict(N_LOC=N_LOC, NBLK=NBLK, NCOL=NCOL, NCH=max(NCH, 1), QS=QS,
                blocks=blocks, MAXCHB=MAXCHB, G_LOC=G_LOC, MAXEB=MAXEB,
                NEBT=max(NEBT, 1), pos_of_b=pos_of_b, e0_b=e0_b,
                wstart_b=wstart_b,
                chunk_wins=[sorted(s) for s in chunk_wins],
                n_graphs_core=n_graphs_core.tolist())
    arrays = dict(xT=xT, dstwE=dstwE, ewE=ewE, idx16=idx16,
                  gid=gid, invn=invn, dis=dis_a, eyedis=eyedis)
    return meta, arrays


def _build_program(meta):
    from concourse import bass, bacc, tile, mybir

    N_LOC, NBLK, NCH = meta['N_LOC'], meta['NBLK'], meta['NCH']
    NCOL, QS = meta['NCOL'], meta['QS']
    blocks, MAXCHB = meta['blocks'], meta['MAXCHB']
    G_LOC, MAXEB, NEBT = meta['G_LOC'], meta['MAXEB'], meta['NEBT']

    nc = bacc.Bacc("TRN2", target_bir_lowering=False, debug=False,
                   num_devices=N_CORES, num_swdge_queues=4)
    f32, bf16, i16 = mybir.dt.float32, mybir.dt.bfloat16, mybir.dt.int16
    AF = mybir.ActivationFunctionType
    OP = mybir.AluOpType

    xT_in = nc.dram_tensor("xT", [128, N_LOC], bf16, kind="ExternalInput")
    dstw_in = nc.dram_tensor("dstwE", [128, NEBT], bf16, kind="ExternalInput")
    ew_in = nc.dram_tensor("ewE", [128, NEBT], bf16, kind="ExternalInput")
    idx_in = nc.dram_tensor("idx16", [128, NCH * 8], i16, kind="ExternalInput")
    gid_in = nc.dram_tensor("gid", [128, NCOL], f32, kind="ExternalInput")
    invn_in = nc.dram_tensor("invn", [128, NCOL], f32, kind="ExternalInput")
    dis_in = nc.dram_tensor("dis", [128, NCOL], f32, kind="ExternalInput")
    eyedis_in = nc.dram_tensor("eyedis", [128, NCOL * 128], bf16,
                               kind="ExternalInput")
    iota_in = nc.dram_tensor("iotaW", [128, WIN], bf16, kind="ExternalInput")
    iotaG_in = nc.dram_tensor("iotaG", [128, G_LOC], f32, kind="ExternalInput")
    eye_in = nc.dram_tensor("eye", [128, 128], bf16, kind="ExternalInput")
    W_in = [nc.dram_tensor(f"W{l}", [128, 128], bf16, kind="ExternalInput") for l in (1, 2, 3)]
    b12_in = [nc.dram_tensor(f"b{l}", [128, 1], f32, kind="ExternalInput") for l in (1, 2, 3)]
    fcw_in = nc.dram_tensor("fcw", [128, OUT_DIM], f32, kind="ExternalInput")
    fcb_in = nc.dram_tensor("fcbrep", [128, OUT_DIM], f32, kind="ExternalInput")
    y_out = nc.dram_tensor("y", [G_LOC, OUT_DIM], f32, kind="ExternalOutput")

    with tile.TileContext(nc) as tc:
        with (
            tc.tile_pool(name="const", bufs=1) as cpool,
            tc.tile_pool(name="big", bufs=1) as bigpool,
            tc.tile_pool(name="gat", bufs=3) as gatpool,
            tc.tile_pool(name="msel", bufs=3) as mselpool,
            tc.tile_pool(name="work", bufs=2) as workpool,
            tc.tile_pool(name="hcol", bufs=4) as hcolpool,
            tc.tile_pool(name="slst", bufs=3) as slpool,
            tc.tile_pool(name="hp", bufs=2, space="PSUM") as hpsum,
            tc.tile_pool(name="sp", bufs=2, space="PSUM") as spsum,
            tc.tile_pool(name="pp", bufs=1, space="PSUM") as ppsum,
            tc.tile_pool(name="dram", bufs=1, space="DRAM") as dpool,
        ):
            def load(shape, src, tag, dt=f32, pool=cpool):
                t = pool.tile(shape, dt, tag=tag)
                nc.sync.dma_start(t[:], src[:])
                return t
            dstw_t = load([128, NEBT], dstw_in, "dstwE", bf16)
            ew_t = load([128, NEBT], ew_in, "ewE", bf16)
            idx_t = load([128, NCH * 8], idx_in, "idx16", i16)
            gid_t = load([128, NCOL], gid_in, "gid")
            invn_t = load([128, NCOL], invn_in, "invn")
            dis_t = load([128, NCOL], dis_in, "dis")
            iota_t = load([128, WIN], iota_in, "iotaW", bf16)
            iotaG_t = load([128, G_LOC], iotaG_in, "iotaG")
            eye_bf = load([128, 128], eye_in, "eye", bf16)
            W_t = [load([128, 128], w, f"W{i}", bf16) for i, w in enumerate(W_in)]
            b12_t = [load([128, 1], b, f"b{i}") for i, b in enumerate(b12_in)]
            fcw_t = load([128, OUT_DIM], fcw_in, "fcw")
            fcb_t = load([128, OUT_DIM], fcb_in, "fcb")

            gbuf = bigpool.tile([128, N_LOC], bf16, tag="gbuf")
            QN = N_LOC // 4              # nodes per quarter table slice
            QCOL = QN // 128
            for j in range(4):           # quarter-granular load: phase A can
                nc.sync.dma_start(       # start before the whole x arrives
                    gbuf[:, j * QN:(j + 1) * QN],
                    xT_in[:, j * QN:(j + 1) * QN])

            # ---- layers (phase A of layer l+1 interleaved into layer l) ----
            qrr = [0]                    # gather queue round-robin counter
            ltabQ_l = [[dpool.tile([QN, 128], bf16, tag=f"ltab{li}q{j}",
                                   name=f"ltab{li}q{j}") for j in range(4)]
                       for li in range(3)]
            tableQ_l = [[dpool.tile([N_CORES * QN, 128], bf16,
                                    tag=f"table{li}q{j}", name=f"table{li}q{j}",
                                    addr_space="Shared")
                         for j in range(4)] for li in range(3)]

            def emit_phaseA_col(li, i):
                hp = hpsum.tile([128, 128], f32, tag="hp")
                nc.tensor.matmul(hp[:], lhsT=gbuf[:, i * 128:(i + 1) * 128],
                                 rhs=W_t[li][:], start=True, stop=True)
                hcol = hcolpool.tile([128, 128], bf16, tag="hcol")
                nc.scalar.activation(hcol[:], hp[:], AF.Copy,
                                     scale=dis_t[:, i:i + 1])
                j, ji = i // QCOL, i % QCOL
                nc.sync.dma_start(ltabQ_l[li][j][ji * 128:(ji + 1) * 128, :],
                                  hcol[:])
                if (i + 1) % QCOL == 0:
                    nc.gpsimd.collective_compute(
                        "AllGather", OP.bypass,
                        replica_groups=[list(range(N_CORES))],
                        ins=[ltabQ_l[li][j].opt()],
                        outs=[tableQ_l[li][j].opt()],
                    )

            for i in range(NCOL):
                emit_phaseA_col(0, i)

            for li in range(3):
                tableQ = tableQ_l[li]

                # epilogue emitted one block late so it never heads the
                # Scalar queue before the next block's work is issued
                pending_epi = []

                def flush_epi():
                    for fn in pending_epi:
                        fn()
                    pending_epi.clear()

                ncols_next = [0]

                def emit_next_phase(nblocks_done, li=li):
                    if li == 2:
                        return
                    target = min(nblocks_done * CPB, NCOL)
                    while ncols_next[0] < target:
                        emit_phaseA_col(li + 1, ncols_next[0])
                        ncols_next[0] += 1

                for b in range(NBLK):
                    bl = blocks[b]
                    gat = gatpool.tile([128, MAXCHB * 128], bf16, tag="gat")
                    for (q, c0, nchg) in bl['groups']:
                        for s0 in range(0, nchg, 8):   # <=1024 idx per gather
                            sn = min(8, nchg - s0)
                            c = c0 + s0
                            rel = c - bl['c0']
                            out_ap = gat[:, rel * 128:(rel + sn) * 128].rearrange(
                                "p (c f) -> p c f", f=128)
                            nc.gpsimd.dma_gather(
                                out_ap, tableQ[q][:],
                                idx_t[:, c * 8:(c + sn) * 8],
                                sn * 128, sn * 128, 128,
                                queue_num=qrr[0] % 4)
                            qrr[0] += 1

                    sp = spsum.tile([128, BLK], f32, tag="sp")
                    # window-contiguous emission: PSUM accumulation groups
                    # must not interleave regions within a bank
                    win_emits = [[] for _ in range(WPB)]
                    for cid in range(bl['c0'], bl['c0'] + bl['chb']):
                        for w in meta['chunk_wins'][cid]:
                            win_emits[w].append(cid)
                    pos_of = meta['pos_of_b'][b]
                    e0 = meta['e0_b'][b]
                    wstart = meta['wstart_b'][b]
                    msall = mselpool.tile([128, MAXEB * WIN], bf16, tag="msall")
                    # build in 4 window-group pieces so early windows' matmuls
                    # start before the whole block's Msel is done
                    for g in range(4):
                        p0, p1 = wstart[g * (WPB // 4)], wstart[(g + 1) * (WPB // 4)]
                        ln = p1 - p0
                        if ln == 0:
                            continue
                        out3 = msall[:, p0 * WIN:p1 * WIN].rearrange(
                            "p (c f) -> p c f", f=WIN)
                        nc.vector.tensor_tensor(
                            out=out3,
                            in0=dstw_t[:, e0 + p0:e0 + p1].unsqueeze(2)
                                .broadcast_to([128, ln, WIN]),
                            in1=iota_t[:].unsqueeze(1)
                                .broadcast_to([128, ln, WIN]),
                            op=OP.is_equal)
                        nc.vector.tensor_tensor(
                            out=out3, in0=out3,
                            in1=ew_t[:, e0 + p0:e0 + p1].unsqueeze(2)
                                .broadcast_to([128, ln, WIN]),
                            op=OP.mult)
                    slst = slpool.tile([128, CPB * 128], bf16, tag="slst")
                    for jj in range(CPB):
                        col = b * CPB + jj
                        qj, ji = col // QCOL, col % QCOL
                        nc.sync.dma_start(
                            slst[:, jj * 128:(jj + 1) * 128],
                            ltabQ_l[li][qj][ji * 128:(ji + 1) * 128, :])
                    edis = slpool.tile([128, CPB * 128], bf16, tag="edis")
                    nc.sync.dma_start(
                        edis[:], eyedis_in[:, b * CPB * 128:(b + 1) * CPB * 128])
                    for j in range(WPB):
                        hblk = slst[:, (j // WPC) * 128:(j // WPC + 1) * 128]
                        wc = win_emits[j]
                        nc.tensor.matmul(
                            sp[:, j * WIN:(j + 1) * WIN],
                            lhsT=hblk,
                            rhs=edis[:, (j // WPC) * 128 + (j % WPC) * WIN:
                                     (j // WPC) * 128 + (j % WPC + 1) * WIN],
                            start=True, stop=(len(wc) == 0),
                            skip_group_check=True)
                        for n, cid in enumerate(wc):
                            p0 = pos_of[(j, cid)]
                            r = cid - bl['c0']
                            nc.tensor.matmul(sp[:, j * WIN:(j + 1) * WIN],
                                             lhsT=gat[:, r * 128:(r + 1) * 128],
                                             rhs=msall[:, p0 * WIN:(p0 + 1) * WIN],
                                             start=False, stop=(n == len(wc) - 1),
                                             skip_group_check=True)

                    def make_epi(b=b, sp=sp, li=li):
                        gslice = gbuf[:, b * BLK:(b + 1) * BLK]
                        nc.scalar.activation(gslice, sp[:], AF.Relu,
                                             bias=b12_t[li][:])
                    flush_epi()
                    emit_next_phase(b)
                    pending_epi.append(make_epi)
                flush_epi()
                emit_next_phase(NBLK)

            # ---- pooling (gbuf holds g3 node-feature-major; transpose per column) ----
            pp = ppsum.tile([128, G_LOC], f32, tag="pp")
            for i in range(NCOL):
                tp = hpsum.tile([128, 128], bf16, tag="hptp")
                nc.tensor.transpose(tp[:], gbuf[:, i * 128:(i + 1) * 128],
                                    eye_bf[:])
                g3n = workpool.tile([128, 128], bf16, tag="g3n")
                nc.scalar.activation(g3n[:], tp[:], AF.Copy)
                P = mselpool.tile([128, G_LOC], bf16, tag="P")
                nc.vector.tensor_scalar(
                    out=P[:], in0=iotaG_t[:], scalar1=gid_t[:, i:i + 1],
                    scalar2=invn_t[:, i:i + 1], op0=OP.is_equal, op1=OP.mult)
                nc.tensor.matmul(pp[:], lhsT=g3n[:], rhs=P[:],
                                 start=(i == 0), stop=(i == NCOL - 1),
                                 skip_group_check=True)
            pooledT = cpool.tile([128, G_LOC], f32, tag="pooledT")
            nc.vector.tensor_copy(pooledT[:], pp[:])

            fp = ppsum.tile([128, OUT_DIM], f32, tag="fc")
            nc.tensor.matmul(fp[:G_LOC, :], lhsT=pooledT[:], rhs=fcw_t[:],
                             start=True, stop=True)
            yt = cpool.tile([128, OUT_DIM], f32, tag="yt")
            nc.vector.tensor_tensor(out=yt[:G_LOC, :], in0=fp[:G_LOC, :],
                                    in1=fcb_t[:G_LOC, :], op=OP.add)
            nc.sync.dma_start(y_out[:], yt[:G_LOC, :])

    nc.compile()
    return nc


def _make_in_maps(meta, arrays, W1, b1, W2, b2, W3, b3, fcW, fcb):
    import ml_dtypes
    G_LOC = meta['G_LOC']
    iotaW = np.broadcast_to(np.arange(WIN, dtype=np.float32),
                            (128, WIN)).astype(ml_dtypes.bfloat16).copy()
    iotaG = np.broadcast_to(np.arange(G_LOC, dtype=np.float32), (128, G_LOC)).copy()
    eye = np.eye(128, dtype=np.float32).astype(ml_dtypes.bfloat16)
    fcbrep = np.broadcast_to(np.asarray(fcb, np.float32), (128, OUT_DIM)).copy()
    common = {
        "iotaW": iotaW, "iotaG": iotaG, "eye": eye,
        "W1": np.asarray(W1, np.float32).astype(ml_dtypes.bfloat16),
        "W2": np.asarray(W2, np.float32).astype(ml_dtypes.bfloat16),
        "W3": np.asarray(W3, np.float32).astype(ml_dtypes.bfloat16),
        "b1": np.asarray(b1, np.float32).reshape(128, 1),
        "b2": np.asarray(b2, np.float32).reshape(128, 1),
        "b3": np.asarray(b3, np.float32).reshape(128, 1),
        "fcw": np.asarray(fcW, np.float32),
        "fcbrep": fcbrep,
    }
    in_maps = []
    for c in range(N_CORES):
        m = dict(common)
        for k in ("xT", "idx16", "gid", "invn", "dis", "eyedis"):
            m[k] = arrays[k][c]
        m["dstwE"] = arrays["dstwE"][c].astype(ml_dtypes.bfloat16)
        m["ewE"] = arrays["ewE"][c].astype(ml_dtypes.bfloat16)
        in_maps.append(m)
    return in_maps


def run(x, edge_index, batch, edge_weight, W1, b1, W2, b2, W3, b3, fcW, fcb,
        n_graphs=512, trace=False):
    from concourse import bass_utils
    meta, arrays = _preprocess(x, edge_index, batch, edge_weight, n_graphs)
    nc = _build_program(meta)
    in_maps = _make_in_maps(meta, arrays, W1, b1, W2, b2, W3, b3, fcW, fcb)
    res = bass_utils.run_bass_kernel_spmd(
        nc, in_maps, core_ids=list(range(N_CORES)), trace=trace)
    ng = meta['n_graphs_core']
    y = np.concatenate([res.results[c]["y"][:ng[c]] for c in range(N_CORES)], axis=0)
    return y.astype(np.float32), res


def kernel(x, edge_index, batch, edge_weight, W1, b1, W2, b2, W3, b3, fcW, fcb):
    y, _ = run(np.asarray(x), np.asarray(edge_index), np.asarray(batch),
               np.asarray(edge_weight), W1, b1, W2, b2, W3, b3, fcW, fcb,
               n_graphs=512, trace=False)
    return y


# revision 23
# speedup vs baseline: 1.3211x; 1.0121x over previous
"""GCN (3-layer + mean-pool + FC) on 8 Trainium2 NeuronCores via Bass.

Self-contained: host-side numpy preprocessing shards nodes (at graph
boundaries) and edges (by destination) across 8 cores, builds one SPMD
Bass program, runs it via run_bass_kernel_spmd, and reassembles the
full [512, 11] output.

Algorithm per GCN layer (h' := dis * (g @ W), dis := rsqrt(deg+1)):
  agg[d] = sum_{e: dst=d} (ew_e * dis_d) * h'[src_e]  +  dis_d * h'[d] + b
computed as dense selection matmuls.  Both dis factors are folded into
host data: ew' = ew * dis_dst scales the edge-selection matrix Msel, and
the self-loop uses rhs = diag(dis) slices, so the epilogue is a single
Relu+bias activation.

Edges are sorted by destination, cut into 128-slot chunks whose
destinations fall inside 32-wide windows of a 512-dst block; per chunk
PSUM accumulates S_T[f, dw] += gathered[slot, f]^T @ Msel[slot, dw]
with Msel[slot, dw] = (iota == dstw[slot]) * ew'[slot] built in two DVE
ops per run, and the self-loop added via diag(dis) matmuls of the local
h' block (which also initialize the PSUM window).

The per-edge source-row gather uses the SWDGE dma_gather instruction:
one gather per (dst-block, table-quarter, <=8 chunks), where the
allgathered bf16 h' table (8*N_LOC rows) is addressed in 2*N_LOC-row
quarters so row indices fit in int16.  Chunks are quarter-pure; chunk
structure is uniform across cores (max over cores per cell) so one SPMD
program serves all 8.
"""
import sys
import numpy as np

sys.path.insert(0, '/opt/trn_rl_repo')

N_CORES = 8
F = 128          # feature dim (in = hid = 128)
OUT_DIM = 11
BLK = 512        # dst nodes per dst-block (psum + epilogue granularity)
WIN = 32         # dst window per chunk matmul
WPB = BLK // WIN
CPB = BLK // 128  # 128-col groups per block
WPC = 128 // WIN  # windows per 128-col group


def _preprocess(x, edge_index, batch, edge_weight, n_graphs):
    """Shard nodes/edges across cores; build per-core device arrays and the
    (core-uniform) chunk structure."""
    import ml_dtypes
    n_nodes = x.shape[0]
    batch = np.asarray(batch).astype(np.int64)
    src = np.asarray(edge_index[0]).astype(np.int64)
    dst = np.asarray(edge_index[1]).astype(np.int64)
    ew = np.asarray(edge_weight).astype(np.float32)

    # --- node shards cut at graph boundaries ---
    gstart = np.searchsorted(batch, np.arange(n_graphs + 1))
    cuts = [0]
    for c in range(1, N_CORES):
        target = c * n_nodes / N_CORES
        g = int(np.searchsorted(gstart, target))
        if g > 0 and target - gstart[g - 1] < gstart[g] - target:
            g = g - 1
        g = min(max(g, cuts[-1]), n_graphs)
        cuts.append(g)
    cuts.append(n_graphs)
    cuts = np.array(cuts)
    node_lo = gstart[cuts[:-1]]
    node_hi = gstart[cuts[1:]]
    n_loc_real = node_hi - node_lo
    N_LOC = int(np.ceil(max(n_loc_real.max(), 1) / BLK) * BLK)
    assert 2 * N_LOC <= 32767, "table quarter must fit int16 indices"
    NBLK = N_LOC // BLK
    NCOL = N_LOC // 128
    QS = 2 * N_LOC           # rows per table quarter
    n_graphs_core = cuts[1:] - cuts[:-1]
    G_LOC = int(n_graphs_core.max())

    owner = np.searchsorted(node_hi, np.arange(n_nodes), side='right')
    local = np.arange(n_nodes) - node_lo[owner]
    # quarter table: node (c, r) lives in quarter r//QN at row c*QN + r%QN
    QN = N_LOC // 4
    q_of = local // QN
    qrow = owner * QN + local % QN      # row within its quarter table

    # --- degree / dis on host (sym normalization with self-loop) ---
    deg_g = np.zeros(n_nodes, np.float64)
    np.add.at(deg_g, dst, ew.astype(np.float64))
    dis_g = (1.0 / np.sqrt(deg_g + 1.0)).astype(np.float32)

    # --- edges assigned by dst; cells = (block, src-quarter) ---
    e_core = owner[dst]
    e_dstl = local[dst]
    e_q = q_of[src]
    e_b = e_dstl // BLK
    NCELL = NBLK * 4
    e_key = e_b * 4 + e_q

    cnt = np.zeros((N_CORES, NCELL), np.int64)
    for c in range(N_CORES):
        np.add.at(cnt[c], e_key[e_core == c], 1)
    nch_cell = np.ceil(cnt.max(axis=0) / 128).astype(np.int64)  # [NCELL]
    cell_off = np.concatenate([[0], np.cumsum(nch_cell)])
    NCH = int(cell_off[-1])

    # program structure (python constants, identical for all cores).
    chunk_wins = [set() for _ in range(NCH)]
    blocks = []
    for b in range(NBLK):
        groups = []
        for q in range(4):
            cell = b * 4 + q
            c0 = int(cell_off[cell])
            nchg = int(cell_off[cell + 1] - c0)
            if nchg > 0:
                groups.append((q, c0, nchg))
        c0b = int(cell_off[b * 4])
        chb = int(cell_off[(b + 1) * 4]) - c0b
        blocks.append(dict(groups=groups, c0=c0b, chb=chb))
    MAXCHB = max(bl['chb'] for bl in blocks) if NCH else 1

    # --- per-core device arrays ---
    dstw = np.zeros((N_CORES, 128, max(NCH, 1)), np.float32)
    ewa = np.zeros((N_CORES, 128, max(NCH, 1)), np.float32)
    idx16 = np.zeros((N_CORES, 128, max(NCH, 1) * 8), np.int16)

    for c in range(N_CORES):
        m = np.where(e_core == c)[0]
        k = e_key[m]
        order = np.lexsort((e_dstl[m], k))   # cell-major, dst minor
        me = m[order]
        ks = k[order]
        cell_start = np.searchsorted(ks, np.arange(NCELL))
        pos = np.arange(len(me)) - cell_start[ks]
        slot = cell_off[ks] * 128 + pos
        cid = slot // 128
        p = slot % 128
        dstw[c, p, cid] = (e_dstl[me] % BLK).astype(np.float32) - 256.0
        ewa[c, p, cid] = ew[me] * dis_g[dst[me]]
        idxv = qrow[src[me]].astype(np.int16)
        col = cid * 8 + p // 16
        for r in range(8):
            idx16[c, 16 * r + p % 16, col] = idxv
        for cw in set(zip(cid.tolist(), ((e_dstl[me] % BLK) // WIN).tolist())):
            chunk_wins[cw[0]].add(cw[1])

    # --- emission positions (window-major per block) + per-emission arrays:
    # dstwE[slot, pos] = dst_local%BLK - j*WIN for emission (window j, chunk
    # cid); matches iotaW = 0..WIN-1 exactly in bf16 (mismatches stay outside
    # [0, WIN)).  ewE is the ew' value replicated per emission of the chunk.
    pos_of_b = []       # per block: {(j, cid): pos}
    e0_b = []           # per block: base emission position
    wstart_b = []       # per block: position-start per window (len WPB+1)
    NEBT = 0
    for b in range(NBLK):
        bl = blocks[b]
        win_emits = [[] for _ in range(WPB)]
        for cid in range(bl['c0'], bl['c0'] + bl['chb']):
            for w in chunk_wins[cid]:
                win_emits[w].append(cid)
        pos_of = {}
        pos = 0
        wstart = []
        for j in range(WPB):
            wstart.append(pos)
            for cid in win_emits[j]:
                pos_of[(j, cid)] = pos
                pos += 1
        wstart.append(pos)
        pos_of_b.append(pos_of)
        wstart_b.append(wstart)
        e0_b.append(NEBT)
        NEBT += pos
    MAXEB = max((len(p) for p in pos_of_b), default=1) or 1
    dstwE = np.full((N_CORES, 128, max(NEBT, 1)), -1.0, np.float32)
    ewE = np.zeros((N_CORES, 128, max(NEBT, 1)), np.float32)
    for b in range(NBLK):
        for (j, cid), pos in pos_of_b[b].items():
            gp = e0_b[b] + pos
            dstwE[:, :, gp] = dstw[:, :, cid] + (256.0 - j * WIN)
            ewE[:, :, gp] = ewa[:, :, cid]
    # padded slots (ew==0) get dstwE=-1 so they never match
    dstwE[ewE == 0.0] = -1.0

    gid = np.full((N_CORES, 128, NCOL), -1.0, np.float32)
    invn = np.ones((N_CORES, 128, NCOL), np.float32)
    dis_a = np.zeros((N_CORES, 128, NCOL), np.float32)
    eyedis = np.zeros((N_CORES, 128, NCOL * 128), ml_dtypes.bfloat16)
    gcnt = np.bincount(batch, minlength=n_graphs).astype(np.float32)
    ar = np.arange(128)
    for c in range(N_CORES):
        n = n_loc_real[c]
        ids = np.arange(node_lo[c], node_hi[c])
        rel = batch[ids] - cuts[c]
        li = np.arange(n)
        gid[c, li % 128, li // 128] = rel.astype(np.float32)
        invn[c, li % 128, li // 128] = 1.0 / gcnt[batch[ids]]
        dis_a[c, li % 128, li // 128] = dis_g[ids]
        for col in range(NCOL):
            eyedis[c, ar, col * 128 + ar] = dis_a[c, :, col].astype(
                ml_dtypes.bfloat16)

    xT = np.zeros((N_CORES, 128, N_LOC), ml_dtypes.bfloat16)
    for c in range(N_CORES):
        n = n_loc_real[c]
        xT[c, :, :n] = np.asarray(x[node_lo[c]:node_hi[c]]).astype(np.float32).T

    meta = dict(N_LOC=N_LOC, NBLK=NBLK, NCOL=NCOL, NCH=max(NCH, 1), QS=QS,
                blocks=blocks, MAXCHB=MAXCHB, G_LOC=G_LOC, MAXEB=MAXEB,
                NEBT=max(NEBT, 1), pos_of_b=pos_of_b, e0_b=e0_b,
                wstart_b=wstart_b,
                chunk_wins=[sorted(s) for s in chunk_wins],
                n_graphs_core=n_graphs_core.tolist())
    arrays = dict(xT=xT, dstwE=dstwE, ewE=ewE, idx16=idx16,
                  gid=gid, invn=invn, dis=dis_a, eyedis=eyedis)
    return meta, arrays


def _build_program(meta):
    from concourse import bass, bacc, tile, mybir

    N_LOC, NBLK, NCH = meta['N_LOC'], meta['NBLK'], meta['NCH']
    NCOL, QS = meta['NCOL'], meta['QS']
    blocks, MAXCHB = meta['blocks'], meta['MAXCHB']
    G_LOC, MAXEB, NEBT = meta['G_LOC'], meta['MAXEB'], meta['NEBT']

    nc = bacc.Bacc("TRN2", target_bir_lowering=False, debug=False,
                   num_devices=N_CORES, num_swdge_queues=4)
    f32, bf16, i16 = mybir.dt.float32, mybir.dt.bfloat16, mybir.dt.int16
    AF = mybir.ActivationFunctionType
    OP = mybir.AluOpType

    xT_in = nc.dram_tensor("xT", [128, N_LOC], bf16, kind="ExternalInput")
    dstw_in = nc.dram_tensor("dstwE", [128, NEBT], bf16, kind="ExternalInput")
    ew_in = nc.dram_tensor("ewE", [128, NEBT], bf16, kind="ExternalInput")
    idx_in = nc.dram_tensor("idx16", [128, NCH * 8], i16, kind="ExternalInput")
    gid_in = nc.dram_tensor("gid", [128, NCOL], f32, kind="ExternalInput")
    invn_in = nc.dram_tensor("invn", [128, NCOL], f32, kind="ExternalInput")
    dis_in = nc.dram_tensor("dis", [128, NCOL], f32, kind="ExternalInput")
    eyedis_in = nc.dram_tensor("eyedis", [128, NCOL * 128], bf16,
                               kind="ExternalInput")
    iota_in = nc.dram_tensor("iotaW", [128, WIN], bf16, kind="ExternalInput")
    iotaG_in = nc.dram_tensor("iotaG", [128, G_LOC], f32, kind="ExternalInput")
    eye_in = nc.dram_tensor("eye", [128, 128], bf16, kind="ExternalInput")
    W_in = [nc.dram_tensor(f"W{l}", [128, 128], bf16, kind="ExternalInput") for l in (1, 2, 3)]
    b12_in = [nc.dram_tensor(f"b{l}", [128, 1], f32, kind="ExternalInput") for l in (1, 2, 3)]
    fcw_in = nc.dram_tensor("fcw", [128, OUT_DIM], f32, kind="ExternalInput")
    fcb_in = nc.dram_tensor("fcbrep", [128, OUT_DIM], f32, kind="ExternalInput")
    y_out = nc.dram_tensor("y", [G_LOC, OUT_DIM], f32, kind="ExternalOutput")

    with tile.TileContext(nc) as tc:
        with (
            tc.tile_pool(name="const", bufs=1) as cpool,
            tc.tile_pool(name="big", bufs=1) as bigpool,
            tc.tile_pool(name="gat", bufs=4) as gatpool,
            tc.tile_pool(name="msel", bufs=3) as mselpool,
            tc.tile_pool(name="work", bufs=2) as workpool,
            tc.tile_pool(name="hcol", bufs=4) as hcolpool,
            tc.tile_pool(name="slst", bufs=3) as slpool,
            tc.tile_pool(name="hp", bufs=2, space="PSUM") as hpsum,
            tc.tile_pool(name="sp", bufs=2, space="PSUM") as spsum,
            tc.tile_pool(name="pp", bufs=1, space="PSUM") as ppsum,
            tc.tile_pool(name="dram", bufs=1, space="DRAM") as dpool,
        ):
            def load(shape, src, tag, dt=f32, pool=cpool):
                t = pool.tile(shape, dt, tag=tag)
                nc.sync.dma_start(t[:], src[:])
                return t
            # phase-A-critical loads first so layer 1 starts ASAP
            gbuf = bigpool.tile([128, N_LOC], bf16, tag="gbuf")
            QN = N_LOC // 4              # nodes per quarter table slice
            QCOL = QN // 128
            for j in range(4):           # quarter-granular load: phase A can
                nc.sync.dma_start(       # start before the whole x arrives
                    gbuf[:, j * QN:(j + 1) * QN],
                    xT_in[:, j * QN:(j + 1) * QN])
            W_t = [load([128, 128], w, f"W{i}", bf16) for i, w in enumerate(W_in)]
            dis_t = load([128, NCOL], dis_in, "dis")
            b12_t = [load([128, 1], b, f"b{i}") for i, b in enumerate(b12_in)]
            idx_t = load([128, NCH * 8], idx_in, "idx16", i16)
            dstw_t = load([128, NEBT], dstw_in, "dstwE", bf16)
            ew_t = load([128, NEBT], ew_in, "ewE", bf16)
            gid_t = load([128, NCOL], gid_in, "gid")
            invn_t = load([128, NCOL], invn_in, "invn")
            iota_t = load([128, WIN], iota_in, "iotaW", bf16)
            iotaG_t = load([128, G_LOC], iotaG_in, "iotaG")
            eye_bf = load([128, 128], eye_in, "eye", bf16)
            fcw_t = load([128, OUT_DIM], fcw_in, "fcw")
            fcb_t = load([128, OUT_DIM], fcb_in, "fcb")

            # ---- layers (phase A of layer l+1 interleaved into layer l) ----
            qrr = [0]                    # gather queue round-robin counter
            ltabQ_l = [[dpool.tile([QN, 128], bf16, tag=f"ltab{li}q{j}",
                                   name=f"ltab{li}q{j}") for j in range(4)]
                       for li in range(3)]
            tableQ_l = [[dpool.tile([N_CORES * QN, 128], bf16,
                                    tag=f"table{li}q{j}", name=f"table{li}q{j}",
                                    addr_space="Shared")
                         for j in range(4)] for li in range(3)]

            def emit_phaseA_col(li, i):
                hp = hpsum.tile([128, 128], f32, tag="hp")
                nc.tensor.matmul(hp[:], lhsT=gbuf[:, i * 128:(i + 1) * 128],
                                 rhs=W_t[li][:], start=True, stop=True)
                hcol = hcolpool.tile([128, 128], bf16, tag="hcol")
                nc.scalar.activation(hcol[:], hp[:], AF.Copy,
                                     scale=dis_t[:, i:i + 1])
                j, ji = i // QCOL, i % QCOL
                nc.sync.dma_start(ltabQ_l[li][j][ji * 128:(ji + 1) * 128, :],
                                  hcol[:])
                if (i + 1) % QCOL == 0:
                    nc.gpsimd.collective_compute(
                        "AllGather", OP.bypass,
                        replica_groups=[list(range(N_CORES))],
                        ins=[ltabQ_l[li][j].opt()],
                        outs=[tableQ_l[li][j].opt()],
                    )

            for i in range(NCOL):
                emit_phaseA_col(0, i)

            for li in range(3):
                tableQ = tableQ_l[li]

                # epilogue emitted one block late so it never heads the
                # Scalar queue before the next block's work is issued
                pending_epi = []

                def flush_epi():
                    for fn in pending_epi:
                        fn()
                    pending_epi.clear()

                ncols_next = [0]

                def emit_next_phase(nblocks_done, li=li):
                    if li == 2:
                        return
                    target = min(nblocks_done * CPB, NCOL)
                    while ncols_next[0] < target:
                        emit_phaseA_col(li + 1, ncols_next[0])
                        ncols_next[0] += 1

                SN = 8   # 1024 idx = 65 descs/engine fits the SWDGE ring

                def emit_gathers(gat, bl, qsel):
                    for (q, c0, nchg) in bl['groups']:
                        if q not in qsel:
                            continue
                        for s0 in range(0, nchg, SN):
                            sn = min(SN, nchg - s0)
                            c = c0 + s0
                            rel = c - bl['c0']
                            out_ap = gat[:, rel * 128:(rel + sn) * 128].rearrange(
                                "p (c f) -> p c f", f=128)
                            nc.gpsimd.dma_gather(
                                out_ap, tableQ[q][:],
                                idx_t[:, c * 8:(c + sn) * 8],
                                sn * 128, sn * 128, 128,
                                queue_num=qrr[0] % 4)
                            qrr[0] += 1

                # head blocks: emit q0..q2 gathers for the first few blocks
                # before any q3 gather, so the queues don't all stall behind
                # the last table quarter's AllGather
                NHEAD = min(3, NBLK)
                head_gats = [gatpool.tile([128, MAXCHB * 128], bf16, tag="gat",
                                          name=f"hgat{li}_{hb}")
                             for hb in range(NHEAD)]
                for q in (0, 1, 2):
                    for hb in range(NHEAD):
                        emit_gathers(head_gats[hb], blocks[hb], (q,))
                for hb in range(NHEAD):
                    emit_gathers(head_gats[hb], blocks[hb], (3,))

                for b in range(NBLK):
                    bl = blocks[b]
                    if b < NHEAD:
                        gat = head_gats[b]
                    else:
                        gat = gatpool.tile([128, MAXCHB * 128], bf16, tag="gat")
                        emit_gathers(gat, bl, (0, 1, 2, 3))

                    sp = spsum.tile([128, BLK], f32, tag="sp")
                    # window-contiguous emission: PSUM accumulation groups
                    # must not interleave regions within a bank
                    win_emits = [[] for _ in range(WPB)]
                    for cid in range(bl['c0'], bl['c0'] + bl['chb']):
                        for w in meta['chunk_wins'][cid]:
                            win_emits[w].append(cid)
                    pos_of = meta['pos_of_b'][b]
                    e0 = meta['e0_b'][b]
                    wstart = meta['wstart_b'][b]
                    msall = mselpool.tile([128, MAXEB * WIN], bf16, tag="msall")
                    # build in 4 window-group pieces so early windows' matmuls
                    # start before the whole block's Msel is done
                    for g in range(4):
                        p0, p1 = wstart[g * (WPB // 4)], wstart[(g + 1) * (WPB // 4)]
                        ln = p1 - p0
                        if ln == 0:
                            continue
                        out3 = msall[:, p0 * WIN:p1 * WIN].rearrange(
                            "p (c f) -> p c f", f=WIN)
                        nc.vector.tensor_tensor(
                            out=out3,
                            in0=dstw_t[:, e0 + p0:e0 + p1].unsqueeze(2)
                                .broadcast_to([128, ln, WIN]),
                            in1=iota_t[:].unsqueeze(1)
                                .broadcast_to([128, ln, WIN]),
                            op=OP.is_equal)
                        nc.vector.tensor_tensor(
                            out=out3, in0=out3,
                            in1=ew_t[:, e0 + p0:e0 + p1].unsqueeze(2)
                                .broadcast_to([128, ln, WIN]),
                            op=OP.mult)
                    slst = slpool.tile([128, CPB * 128], bf16, tag="slst")
                    for jj in range(CPB):
                        col = b * CPB + jj
                        qj, ji = col // QCOL, col % QCOL
                        nc.sync.dma_start(
                            slst[:, jj * 128:(jj + 1) * 128],
                            ltabQ_l[li][qj][ji * 128:(ji + 1) * 128, :])
                    edis = slpool.tile([128, CPB * 128], bf16, tag="edis")
                    nc.sync.dma_start(
                        edis[:], eyedis_in[:, b * CPB * 128:(b + 1) * CPB * 128])
                    for j in range(WPB):
                        hblk = slst[:, (j // WPC) * 128:(j // WPC + 1) * 128]
                        wc = win_emits[j]
                        nc.tensor.matmul(
                            sp[:, j * WIN:(j + 1) * WIN],
                            lhsT=hblk,
                            rhs=edis[:, (j // WPC) * 128 + (j % WPC) * WIN:
                                     (j // WPC) * 128 + (j % WPC + 1) * WIN],
                            start=True, stop=(len(wc) == 0),
                            skip_group_check=True)
                        for n, cid in enumerate(wc):
                            p0 = pos_of[(j, cid)]
                            r = cid - bl['c0']
                            nc.tensor.matmul(sp[:, j * WIN:(j + 1) * WIN],
                                             lhsT=gat[:, r * 128:(r + 1) * 128],
                                             rhs=msall[:, p0 * WIN:(p0 + 1) * WIN],
                                             start=False, stop=(n == len(wc) - 1),
                                             skip_group_check=True)

                    def make_epi(b=b, sp=sp, li=li):
                        gslice = gbuf[:, b * BLK:(b + 1) * BLK]
                        nc.scalar.activation(gslice, sp[:], AF.Relu,
                                             bias=b12_t[li][:])
                    flush_epi()
                    emit_next_phase(b)
                    pending_epi.append(make_epi)
                flush_epi()
                emit_next_phase(NBLK)

            # ---- pooling (gbuf holds g3 node-feature-major; transpose per column) ----
            pp = ppsum.tile([128, G_LOC], f32, tag="pp")
            for i in range(NCOL):
                tp = hpsum.tile([128, 128], bf16, tag="hptp")
                nc.tensor.transpose(tp[:], gbuf[:, i * 128:(i + 1) * 128],
                                    eye_bf[:])
                g3n = workpool.tile([128, 128], bf16, tag="g3n")
                nc.scalar.activation(g3n[:], tp[:], AF.Copy)
                P = mselpool.tile([128, G_LOC], bf16, tag="P")
                nc.vector.tensor_scalar(
                    out=P[:], in0=iotaG_t[:], scalar1=gid_t[:, i:i + 1],
                    scalar2=invn_t[:, i:i + 1], op0=OP.is_equal, op1=OP.mult)
                nc.tensor.matmul(pp[:], lhsT=g3n[:], rhs=P[:],
                                 start=(i == 0), stop=(i == NCOL - 1),
                                 skip_group_check=True)
            pooledT = cpool.tile([128, G_LOC], f32, tag="pooledT")
            nc.vector.tensor_copy(pooledT[:], pp[:])

            fp = ppsum.tile([128, OUT_DIM], f32, tag="fc")
            nc.tensor.matmul(fp[:G_LOC, :], lhsT=pooledT[:], rhs=fcw_t[:],
                             start=True, stop=True)
            yt = cpool.tile([128, OUT_DIM], f32, tag="yt")
            nc.vector.tensor_tensor(out=yt[:G_LOC, :], in0=fp[:G_LOC, :],
                                    in1=fcb_t[:G_LOC, :], op=OP.add)
            nc.sync.dma_start(y_out[:], yt[:G_LOC, :])

    nc.compile()
    return nc


def _make_in_maps(meta, arrays, W1, b1, W2, b2, W3, b3, fcW, fcb):
    import ml_dtypes
    G_LOC = meta['G_LOC']
    iotaW = np.broadcast_to(np.arange(WIN, dtype=np.float32),
                            (128, WIN)).astype(ml_dtypes.bfloat16).copy()
    iotaG = np.broadcast_to(np.arange(G_LOC, dtype=np.float32), (128, G_LOC)).copy()
    eye = np.eye(128, dtype=np.float32).astype(ml_dtypes.bfloat16)
    fcbrep = np.broadcast_to(np.asarray(fcb, np.float32), (128, OUT_DIM)).copy()
    common = {
        "iotaW": iotaW, "iotaG": iotaG, "eye": eye,
        "W1": np.asarray(W1, np.float32).astype(ml_dtypes.bfloat16),
        "W2": np.asarray(W2, np.float32).astype(ml_dtypes.bfloat16),
        "W3": np.asarray(W3, np.float32).astype(ml_dtypes.bfloat16),
        "b1": np.asarray(b1, np.float32).reshape(128, 1),
        "b2": np.asarray(b2, np.float32).reshape(128, 1),
        "b3": np.asarray(b3, np.float32).reshape(128, 1),
        "fcw": np.asarray(fcW, np.float32),
        "fcbrep": fcbrep,
    }
    in_maps = []
    for c in range(N_CORES):
        m = dict(common)
        for k in ("xT", "idx16", "gid", "invn", "dis", "eyedis"):
            m[k] = arrays[k][c]
        m["dstwE"] = arrays["dstwE"][c].astype(ml_dtypes.bfloat16)
        m["ewE"] = arrays["ewE"][c].astype(ml_dtypes.bfloat16)
        in_maps.append(m)
    return in_maps


def run(x, edge_index, batch, edge_weight, W1, b1, W2, b2, W3, b3, fcW, fcb,
        n_graphs=512, trace=False):
    from concourse import bass_utils
    meta, arrays = _preprocess(x, edge_index, batch, edge_weight, n_graphs)
    nc = _build_program(meta)
    in_maps = _make_in_maps(meta, arrays, W1, b1, W2, b2, W3, b3, fcW, fcb)
    res = bass_utils.run_bass_kernel_spmd(
        nc, in_maps, core_ids=list(range(N_CORES)), trace=trace)
    ng = meta['n_graphs_core']
    y = np.concatenate([res.results[c]["y"][:ng[c]] for c in range(N_CORES)], axis=0)
    return y.astype(np.float32), res


def kernel(x, edge_index, batch, edge_weight, W1, b1, W2, b2, W3, b3, fcW, fcb):
    y, _ = run(np.asarray(x), np.asarray(edge_index), np.asarray(batch),
               np.asarray(edge_weight), W1, b1, W2, b2, W3, b3, fcW, fcb,
               n_graphs=512, trace=False)
    return y


# revision 24
# speedup vs baseline: 1.3875x; 1.0503x over previous
"""GCN (3-layer + mean-pool + FC) on 8 Trainium2 NeuronCores via Bass.

Self-contained: host-side numpy preprocessing shards nodes (at graph
boundaries) and edges (by destination) across 8 cores, builds one SPMD
Bass program, runs it via run_bass_kernel_spmd, and reassembles the
full [512, 11] output.

Algorithm per GCN layer (h' := dis * (g @ W), dis := rsqrt(deg+1)):
  agg[d] = sum_{e: dst=d} (ew_e * dis_d) * h'[src_e]  +  dis_d * h'[d] + b
computed as dense selection matmuls.  Both dis factors are folded into
host data: ew' = ew * dis_dst scales the edge-selection matrix Msel, and
the self-loop uses rhs = diag(dis) slices, so the epilogue is a single
Relu+bias activation.

Edges are sorted by destination, cut into 128-slot chunks whose
destinations fall inside 32-wide windows of a 512-dst block; per chunk
PSUM accumulates S_T[f, dw] += gathered[slot, f]^T @ Msel[slot, dw]
with Msel[slot, dw] = (iota == dstw[slot]) * ew'[slot] built in two DVE
ops per run, and the self-loop added via diag(dis) matmuls of the local
h' block (which also initialize the PSUM window).

The per-edge source-row gather uses the SWDGE dma_gather instruction:
one gather per (dst-block, table-quarter, <=8 chunks), where the
allgathered bf16 h' table (8*N_LOC rows) is addressed in 2*N_LOC-row
quarters so row indices fit in int16.  Chunks are quarter-pure; chunk
structure is uniform across cores (max over cores per cell) so one SPMD
program serves all 8.
"""
import sys
import numpy as np

sys.path.insert(0, '/opt/trn_rl_repo')

N_CORES = 8
F = 128          # feature dim (in = hid = 128)
OUT_DIM = 11
BLK = 512        # dst nodes per dst-block (psum + epilogue granularity)
WIN = 32         # dst window per chunk matmul
WPB = BLK // WIN
CPB = BLK // 128  # 128-col groups per block
WPC = 128 // WIN  # windows per 128-col group


def _preprocess(x, edge_index, batch, edge_weight, n_graphs):
    """Shard nodes/edges across cores; build per-core device arrays and the
    (core-uniform) chunk structure."""
    import ml_dtypes
    n_nodes = x.shape[0]
    batch = np.asarray(batch).astype(np.int64)
    src = np.asarray(edge_index[0]).astype(np.int64)
    dst = np.asarray(edge_index[1]).astype(np.int64)
    ew = np.asarray(edge_weight).astype(np.float32)

    # --- node shards cut at graph boundaries ---
    gstart = np.searchsorted(batch, np.arange(n_graphs + 1))
    cuts = [0]
    for c in range(1, N_CORES):
        target = c * n_nodes / N_CORES
        g = int(np.searchsorted(gstart, target))
        if g > 0 and target - gstart[g - 1] < gstart[g] - target:
            g = g - 1
        g = min(max(g, cuts[-1]), n_graphs)
        cuts.append(g)
    cuts.append(n_graphs)
    cuts = np.array(cuts)
    node_lo = gstart[cuts[:-1]]
    node_hi = gstart[cuts[1:]]
    n_loc_real = node_hi - node_lo
    N_LOC = int(np.ceil(max(n_loc_real.max(), 1) / BLK) * BLK)
    assert 2 * N_LOC <= 32767, "table quarter must fit int16 indices"
    NBLK = N_LOC // BLK
    NCOL = N_LOC // 128
    QS = 2 * N_LOC           # rows per table quarter
    n_graphs_core = cuts[1:] - cuts[:-1]
    G_LOC = int(n_graphs_core.max())

    owner = np.searchsorted(node_hi, np.arange(n_nodes), side='right')
    local = np.arange(n_nodes) - node_lo[owner]
    # quarter table: node (c, r) lives in quarter r//QN at row c*QN + r%QN
    QN = N_LOC // 4
    q_of = local // QN
    qrow = owner * QN + local % QN      # row within its quarter table

    # --- degree / dis on host (sym normalization with self-loop) ---
    deg_g = np.zeros(n_nodes, np.float64)
    np.add.at(deg_g, dst, ew.astype(np.float64))
    dis_g = (1.0 / np.sqrt(deg_g + 1.0)).astype(np.float32)

    # --- edges assigned by dst; cells = (block, src-quarter) ---
    e_core = owner[dst]
    e_dstl = local[dst]
    e_q = q_of[src]
    e_b = e_dstl // BLK
    NCELL = NBLK * 4
    e_key = e_b * 4 + e_q

    cnt = np.zeros((N_CORES, NCELL), np.int64)
    for c in range(N_CORES):
        np.add.at(cnt[c], e_key[e_core == c], 1)
    nch_cell = np.ceil(cnt.max(axis=0) / 128).astype(np.int64)  # [NCELL]
    cell_off = np.concatenate([[0], np.cumsum(nch_cell)])
    NCH = int(cell_off[-1])

    # program structure (python constants, identical for all cores).
    chunk_wins = [set() for _ in range(NCH)]
    blocks = []
    for b in range(NBLK):
        groups = []
        for q in range(4):
            cell = b * 4 + q
            c0 = int(cell_off[cell])
            nchg = int(cell_off[cell + 1] - c0)
            if nchg > 0:
                groups.append((q, c0, nchg))
        c0b = int(cell_off[b * 4])
        chb = int(cell_off[(b + 1) * 4]) - c0b
        blocks.append(dict(groups=groups, c0=c0b, chb=chb))
    MAXCHB = max(bl['chb'] for bl in blocks) if NCH else 1

    # --- per-core device arrays ---
    dstw = np.zeros((N_CORES, 128, max(NCH, 1)), np.float32)
    ewa = np.zeros((N_CORES, 128, max(NCH, 1)), np.float32)
    idx16 = np.zeros((N_CORES, 128, max(NCH, 1) * 8), np.int16)

    for c in range(N_CORES):
        m = np.where(e_core == c)[0]
        k = e_key[m]
        order = np.lexsort((e_dstl[m], k))   # cell-major, dst minor
        me = m[order]
        ks = k[order]
        cell_start = np.searchsorted(ks, np.arange(NCELL))
        pos = np.arange(len(me)) - cell_start[ks]
        slot = cell_off[ks] * 128 + pos
        cid = slot // 128
        p = slot % 128
        dstw[c, p, cid] = (e_dstl[me] % BLK).astype(np.float32) - 256.0
        ewa[c, p, cid] = ew[me] * dis_g[dst[me]]
        idxv = qrow[src[me]].astype(np.int16)
        col = cid * 8 + p // 16
        for r in range(8):
            idx16[c, 16 * r + p % 16, col] = idxv
        for cw in set(zip(cid.tolist(), ((e_dstl[me] % BLK) // WIN).tolist())):
            chunk_wins[cw[0]].add(cw[1])

    # --- emission positions (window-major per block) + per-emission arrays:
    # dstwE[slot, pos] = dst_local%BLK - j*WIN for emission (window j, chunk
    # cid); matches iotaW = 0..WIN-1 exactly in bf16 (mismatches stay outside
    # [0, WIN)).  ewE is the ew' value replicated per emission of the chunk.
    pos_of_b = []       # per block: {(j, cid): pos}
    e0_b = []           # per block: base emission position
    wstart_b = []       # per block: position-start per window (len WPB+1)
    NEBT = 0
    for b in range(NBLK):
        bl = blocks[b]
        win_emits = [[] for _ in range(WPB)]
        for cid in range(bl['c0'], bl['c0'] + bl['chb']):
            for w in chunk_wins[cid]:
                win_emits[w].append(cid)
        pos_of = {}
        pos = 0
        wstart = []
        for j in range(WPB):
            wstart.append(pos)
            for cid in win_emits[j]:
                pos_of[(j, cid)] = pos
                pos += 1
        wstart.append(pos)
        pos_of_b.append(pos_of)
        wstart_b.append(wstart)
        e0_b.append(NEBT)
        NEBT += pos
    MAXEB = max((len(p) for p in pos_of_b), default=1) or 1
    dstwE = np.full((N_CORES, 128, max(NEBT, 1)), -1.0, np.float32)
    ewE = np.zeros((N_CORES, 128, max(NEBT, 1)), np.float32)
    for b in range(NBLK):
        for (j, cid), pos in pos_of_b[b].items():
            gp = e0_b[b] + pos
            dstwE[:, :, gp] = dstw[:, :, cid] + (256.0 - j * WIN)
            ewE[:, :, gp] = ewa[:, :, cid]
    # padded slots (ew==0) get dstwE=-1 so they never match
    dstwE[ewE == 0.0] = -1.0

    gid = np.full((N_CORES, 128, NCOL), -1.0, np.float32)
    invn = np.ones((N_CORES, 128, NCOL), np.float32)
    dis_a = np.zeros((N_CORES, 128, NCOL), np.float32)
    eyedis = np.zeros((N_CORES, 128, NCOL * 128), ml_dtypes.bfloat16)
    gcnt = np.bincount(batch, minlength=n_graphs).astype(np.float32)
    ar = np.arange(128)
    for c in range(N_CORES):
        n = n_loc_real[c]
        ids = np.arange(node_lo[c], node_hi[c])
        rel = batch[ids] - cuts[c]
        li = np.arange(n)
        gid[c, li % 128, li // 128] = rel.astype(np.float32)
        invn[c, li % 128, li // 128] = 1.0 / gcnt[batch[ids]]
        dis_a[c, li % 128, li // 128] = dis_g[ids]
        for col in range(NCOL):
            eyedis[c, ar, col * 128 + ar] = dis_a[c, :, col].astype(
                ml_dtypes.bfloat16)

    xT = np.zeros((N_CORES, 128, N_LOC), ml_dtypes.bfloat16)
    for c in range(N_CORES):
        n = n_loc_real[c]
        xT[c, :, :n] = np.asarray(x[node_lo[c]:node_hi[c]]).astype(np.float32).T

    meta = dict(N_LOC=N_LOC, NBLK=NBLK, NCOL=NCOL, NCH=max(NCH, 1), QS=QS,
                blocks=blocks, MAXCHB=MAXCHB, G_LOC=G_LOC, MAXEB=MAXEB,
                NEBT=max(NEBT, 1), pos_of_b=pos_of_b, e0_b=e0_b,
                wstart_b=wstart_b,
                chunk_wins=[sorted(s) for s in chunk_wins],
                n_graphs_core=n_graphs_core.tolist())
    arrays = dict(xT=xT, dstwE=dstwE, ewE=ewE, idx16=idx16,
                  gid=gid, invn=invn, dis=dis_a, eyedis=eyedis)
    return meta, arrays


def _build_program(meta):
    from concourse import bass, bacc, tile, mybir

    N_LOC, NBLK, NCH = meta['N_LOC'], meta['NBLK'], meta['NCH']
    NCOL, QS = meta['NCOL'], meta['QS']
    blocks, MAXCHB = meta['blocks'], meta['MAXCHB']
    G_LOC, MAXEB, NEBT = meta['G_LOC'], meta['MAXEB'], meta['NEBT']

    nc = bacc.Bacc("TRN2", target_bir_lowering=False, debug=False,
                   num_devices=N_CORES, num_swdge_queues=4)
    f32, bf16, i16 = mybir.dt.float32, mybir.dt.bfloat16, mybir.dt.int16
    AF = mybir.ActivationFunctionType
    OP = mybir.AluOpType

    xT_in = nc.dram_tensor("xT", [128, N_LOC], bf16, kind="ExternalInput")
    dstw_in = nc.dram_tensor("dstwE", [128, NEBT], bf16, kind="ExternalInput")
    ew_in = nc.dram_tensor("ewE", [128, NEBT], bf16, kind="ExternalInput")
    idx_in = nc.dram_tensor("idx16", [128, NCH * 8], i16, kind="ExternalInput")
    gid_in = nc.dram_tensor("gid", [128, NCOL], f32, kind="ExternalInput")
    invn_in = nc.dram_tensor("invn", [128, NCOL], f32, kind="ExternalInput")
    dis_in = nc.dram_tensor("dis", [128, NCOL], f32, kind="ExternalInput")
    eyedis_in = nc.dram_tensor("eyedis", [128, NCOL * 128], bf16,
                               kind="ExternalInput")
    iota_in = nc.dram_tensor("iotaW", [128, WIN], bf16, kind="ExternalInput")
    iotaG_in = nc.dram_tensor("iotaG", [128, G_LOC], f32, kind="ExternalInput")
    eye_in = nc.dram_tensor("eye", [128, 128], bf16, kind="ExternalInput")
    W_in = [nc.dram_tensor(f"W{l}", [128, 128], bf16, kind="ExternalInput") for l in (1, 2, 3)]
    b12_in = [nc.dram_tensor(f"b{l}", [128, 1], f32, kind="ExternalInput") for l in (1, 2, 3)]
    fcw_in = nc.dram_tensor("fcw", [128, OUT_DIM], f32, kind="ExternalInput")
    fcb_in = nc.dram_tensor("fcbrep", [128, OUT_DIM], f32, kind="ExternalInput")
    y_out = nc.dram_tensor("y", [G_LOC, OUT_DIM], f32, kind="ExternalOutput")

    with tile.TileContext(nc) as tc:
        with (
            tc.tile_pool(name="const", bufs=1) as cpool,
            tc.tile_pool(name="big", bufs=1) as bigpool,
            tc.tile_pool(name="gat", bufs=4) as gatpool,
            tc.tile_pool(name="msel", bufs=3) as mselpool,
            tc.tile_pool(name="work", bufs=2) as workpool,
            tc.tile_pool(name="hcol", bufs=4) as hcolpool,
            tc.tile_pool(name="slst", bufs=3) as slpool,
            tc.tile_pool(name="hp", bufs=2, space="PSUM") as hpsum,
            tc.tile_pool(name="sp", bufs=2, space="PSUM") as spsum,
            tc.tile_pool(name="pp", bufs=1, space="PSUM") as ppsum,
            tc.tile_pool(name="dram", bufs=1, space="DRAM") as dpool,
        ):
            def load(shape, src, tag, dt=f32, pool=cpool):
                t = pool.tile(shape, dt, tag=tag)
                nc.sync.dma_start(t[:], src[:])
                return t
            # phase-A-critical loads first so layer 1 starts ASAP
            gbuf = bigpool.tile([128, N_LOC], bf16, tag="gbuf")
            QN = N_LOC // 4              # nodes per quarter table slice
            QCOL = QN // 128
            for j in range(4):           # quarter-granular load: phase A can
                nc.sync.dma_start(       # start before the whole x arrives
                    gbuf[:, j * QN:(j + 1) * QN],
                    xT_in[:, j * QN:(j + 1) * QN])
            W_t = [load([128, 128], w, f"W{i}", bf16) for i, w in enumerate(W_in)]
            dis_t = load([128, NCOL], dis_in, "dis")
            b12_t = [load([128, 1], b, f"b{i}") for i, b in enumerate(b12_in)]
            idx_t = load([128, NCH * 8], idx_in, "idx16", i16)
            dstw_t = load([128, NEBT], dstw_in, "dstwE", bf16)
            ew_t = load([128, NEBT], ew_in, "ewE", bf16)
            gid_t = load([128, NCOL], gid_in, "gid")
            invn_t = load([128, NCOL], invn_in, "invn")
            iota_t = load([128, WIN], iota_in, "iotaW", bf16)
            iotaG_t = load([128, G_LOC], iotaG_in, "iotaG")
            eye_bf = load([128, 128], eye_in, "eye", bf16)
            fcw_t = load([128, OUT_DIM], fcw_in, "fcw")
            fcb_t = load([128, OUT_DIM], fcb_in, "fcb")

            # ---- layers (phase A of layer l+1 interleaved into layer l) ----
            qrr = [0]                    # gather queue round-robin counter
            ltabQ_l = [[dpool.tile([QN, 128], bf16, tag=f"ltab{li}q{j}",
                                   name=f"ltab{li}q{j}") for j in range(4)]
                       for li in range(3)]
            tableQ_l = [[dpool.tile([N_CORES * QN, 128], bf16,
                                    tag=f"table{li}q{j}", name=f"table{li}q{j}",
                                    addr_space="Shared")
                         for j in range(4)] for li in range(3)]

            def emit_phaseA_col(li, i):
                hp = hpsum.tile([128, 128], f32, tag="hp")
                nc.tensor.matmul(hp[:], lhsT=gbuf[:, i * 128:(i + 1) * 128],
                                 rhs=W_t[li][:], start=True, stop=True)
                hcol = hcolpool.tile([128, 128], bf16, tag="hcol")
                nc.scalar.activation(hcol[:], hp[:], AF.Copy,
                                     scale=dis_t[:, i:i + 1])
                j, ji = i // QCOL, i % QCOL
                nc.sync.dma_start(ltabQ_l[li][j][ji * 128:(ji + 1) * 128, :],
                                  hcol[:])
                if (i + 1) % QCOL == 0:
                    nc.gpsimd.collective_compute(
                        "AllGather", OP.bypass,
                        replica_groups=[list(range(N_CORES))],
                        ins=[ltabQ_l[li][j].opt()],
                        outs=[tableQ_l[li][j].opt()],
                    )

            for i in range(NCOL):
                emit_phaseA_col(0, i)

            for li in range(3):
                tableQ = tableQ_l[li]

                # epilogue emitted one block late so it never heads the
                # Scalar queue before the next block's work is issued
                pending_epi = []

                def flush_epi():
                    for fn in pending_epi:
                        fn()
                    pending_epi.clear()

                ncols_next = [0]

                def emit_next_phase(nblocks_done, li=li):
                    if li == 2:
                        return
                    target = min(nblocks_done * CPB, NCOL)
                    while ncols_next[0] < target:
                        emit_phaseA_col(li + 1, ncols_next[0])
                        ncols_next[0] += 1

                SN = 8   # 1024 idx = 65 descs/engine fits the SWDGE ring
                qload = [0, 0, 0, 0]

                def emit_gathers(gat, bl, qsel):
                    for (q, c0, nchg) in bl['groups']:
                        if q not in qsel:
                            continue
                        # balanced split: parts differ by <=1 chunk
                        nparts = (nchg + SN - 1) // SN
                        base, ext = divmod(nchg, nparts)
                        s0 = 0
                        for pi in range(nparts):
                            sn = base + (1 if pi < ext else 0)
                            c = c0 + s0
                            rel = c - bl['c0']
                            out_ap = gat[:, rel * 128:(rel + sn) * 128].rearrange(
                                "p (c f) -> p c f", f=128)
                            qn = min(range(4), key=lambda k: qload[k])
                            qload[qn] += sn
                            nc.gpsimd.dma_gather(
                                out_ap, tableQ[q][:],
                                idx_t[:, c * 8:(c + sn) * 8],
                                sn * 128, sn * 128, 128,
                                queue_num=qn)
                            s0 += sn

                # head blocks: emit q0..q2 gathers for the first few blocks
                # before any q3 gather, so the queues don't all stall behind
                # the last table quarter's AllGather
                NHEAD = min(3, NBLK)
                head_gats = [gatpool.tile([128, MAXCHB * 128], bf16, tag="gat",
                                          name=f"hgat{li}_{hb}")
                             for hb in range(NHEAD)]
                for q in (0, 1, 2):
                    for hb in range(NHEAD):
                        emit_gathers(head_gats[hb], blocks[hb], (q,))
                for hb in range(NHEAD):
                    emit_gathers(head_gats[hb], blocks[hb], (3,))

                for b in range(NBLK):
                    bl = blocks[b]
                    if b < NHEAD:
                        gat = head_gats[b]
                    else:
                        gat = gatpool.tile([128, MAXCHB * 128], bf16, tag="gat")
                        emit_gathers(gat, bl, (0, 1, 2, 3))

                    sp = spsum.tile([128, BLK], f32, tag="sp")
                    # window-contiguous emission: PSUM accumulation groups
                    # must not interleave regions within a bank
                    win_emits = [[] for _ in range(WPB)]
                    for cid in range(bl['c0'], bl['c0'] + bl['chb']):
                        for w in meta['chunk_wins'][cid]:
                            win_emits[w].append(cid)
                    pos_of = meta['pos_of_b'][b]
                    e0 = meta['e0_b'][b]
                    wstart = meta['wstart_b'][b]
                    msall = mselpool.tile([128, MAXEB * WIN], bf16, tag="msall")
                    # build in 4 window-group pieces so early windows' matmuls
                    # start before the whole block's Msel is done
                    for g in range(4):
                        p0, p1 = wstart[g * (WPB // 4)], wstart[(g + 1) * (WPB // 4)]
                        ln = p1 - p0
                        if ln == 0:
                            continue
                        out3 = msall[:, p0 * WIN:p1 * WIN].rearrange(
                            "p (c f) -> p c f", f=WIN)
                        nc.vector.tensor_tensor(
                            out=out3,
                            in0=dstw_t[:, e0 + p0:e0 + p1].unsqueeze(2)
                                .broadcast_to([128, ln, WIN]),
                            in1=iota_t[:].unsqueeze(1)
                                .broadcast_to([128, ln, WIN]),
                            op=OP.is_equal)
                        nc.vector.tensor_tensor(
                            out=out3, in0=out3,
                            in1=ew_t[:, e0 + p0:e0 + p1].unsqueeze(2)
                                .broadcast_to([128, ln, WIN]),
                            op=OP.mult)
                    slst = slpool.tile([128, CPB * 128], bf16, tag="slst")
                    for jj in range(CPB):
                        col = b * CPB + jj
                        qj, ji = col // QCOL, col % QCOL
                        nc.sync.dma_start(
                            slst[:, jj * 128:(jj + 1) * 128],
                            ltabQ_l[li][qj][ji * 128:(ji + 1) * 128, :])
                    edis = slpool.tile([128, CPB * 128], bf16, tag="edis")
                    nc.sync.dma_start(
                        edis[:], eyedis_in[:, b * CPB * 128:(b + 1) * CPB * 128])
                    for j in range(WPB):
                        hblk = slst[:, (j // WPC) * 128:(j // WPC + 1) * 128]
                        wc = win_emits[j]
                        nc.tensor.matmul(
                            sp[:, j * WIN:(j + 1) * WIN],
                            lhsT=hblk,
                            rhs=edis[:, (j // WPC) * 128 + (j % WPC) * WIN:
                                     (j // WPC) * 128 + (j % WPC + 1) * WIN],
                            start=True, stop=(len(wc) == 0),
                            skip_group_check=True)
                        for n, cid in enumerate(wc):
                            p0 = pos_of[(j, cid)]
                            r = cid - bl['c0']
                            nc.tensor.matmul(sp[:, j * WIN:(j + 1) * WIN],
                                             lhsT=gat[:, r * 128:(r + 1) * 128],
                                             rhs=msall[:, p0 * WIN:(p0 + 1) * WIN],
                                             start=False, stop=(n == len(wc) - 1),
                                             skip_group_check=True)

                    def make_epi(b=b, sp=sp, li=li):
                        gslice = gbuf[:, b * BLK:(b + 1) * BLK]
                        nc.scalar.activation(gslice, sp[:], AF.Relu,
                                             bias=b12_t[li][:])
                    flush_epi()
                    emit_next_phase(b)
                    pending_epi.append(make_epi)
                flush_epi()
                emit_next_phase(NBLK)

            # ---- pooling (gbuf holds g3 node-feature-major; transpose per column) ----
            pp = ppsum.tile([128, G_LOC], f32, tag="pp")
            for i in range(NCOL):
                tp = hpsum.tile([128, 128], bf16, tag="hptp")
                nc.tensor.transpose(tp[:], gbuf[:, i * 128:(i + 1) * 128],
                                    eye_bf[:])
                g3n = workpool.tile([128, 128], bf16, tag="g3n")
                nc.scalar.activation(g3n[:], tp[:], AF.Copy)
                P = mselpool.tile([128, G_LOC], bf16, tag="P")
                nc.vector.tensor_scalar(
                    out=P[:], in0=iotaG_t[:], scalar1=gid_t[:, i:i + 1],
                    scalar2=invn_t[:, i:i + 1], op0=OP.is_equal, op1=OP.mult)
                nc.tensor.matmul(pp[:], lhsT=g3n[:], rhs=P[:],
                                 start=(i == 0), stop=(i == NCOL - 1),
                                 skip_group_check=True)
            pooledT = cpool.tile([128, G_LOC], f32, tag="pooledT")
            nc.vector.tensor_copy(pooledT[:], pp[:])

            fp = ppsum.tile([128, OUT_DIM], f32, tag="fc")
            nc.tensor.matmul(fp[:G_LOC, :], lhsT=pooledT[:], rhs=fcw_t[:],
                             start=True, stop=True)
            yt = cpool.tile([128, OUT_DIM], f32, tag="yt")
            nc.vector.tensor_tensor(out=yt[:G_LOC, :], in0=fp[:G_LOC, :],
                                    in1=fcb_t[:G_LOC, :], op=OP.add)
            nc.sync.dma_start(y_out[:], yt[:G_LOC, :])

    nc.compile()
    return nc


def _make_in_maps(meta, arrays, W1, b1, W2, b2, W3, b3, fcW, fcb):
    import ml_dtypes
    G_LOC = meta['G_LOC']
    iotaW = np.broadcast_to(np.arange(WIN, dtype=np.float32),
                            (128, WIN)).astype(ml_dtypes.bfloat16).copy()
    iotaG = np.broadcast_to(np.arange(G_LOC, dtype=np.float32), (128, G_LOC)).copy()
    eye = np.eye(128, dtype=np.float32).astype(ml_dtypes.bfloat16)
    fcbrep = np.broadcast_to(np.asarray(fcb, np.float32), (128, OUT_DIM)).copy()
    common = {
        "iotaW": iotaW, "iotaG": iotaG, "eye": eye,
        "W1": np.asarray(W1, np.float32).astype(ml_dtypes.bfloat16),
        "W2": np.asarray(W2, np.float32).astype(ml_dtypes.bfloat16),
        "W3": np.asarray(W3, np.float32).astype(ml_dtypes.bfloat16),
        "b1": np.asarray(b1, np.float32).reshape(128, 1),
        "b2": np.asarray(b2, np.float32).reshape(128, 1),
        "b3": np.asarray(b3, np.float32).reshape(128, 1),
        "fcw": np.asarray(fcW, np.float32),
        "fcbrep": fcbrep,
    }
    in_maps = []
    for c in range(N_CORES):
        m = dict(common)
        for k in ("xT", "idx16", "gid", "invn", "dis", "eyedis"):
            m[k] = arrays[k][c]
        m["dstwE"] = arrays["dstwE"][c].astype(ml_dtypes.bfloat16)
        m["ewE"] = arrays["ewE"][c].astype(ml_dtypes.bfloat16)
        in_maps.append(m)
    return in_maps


def run(x, edge_index, batch, edge_weight, W1, b1, W2, b2, W3, b3, fcW, fcb,
        n_graphs=512, trace=False):
    from concourse import bass_utils
    meta, arrays = _preprocess(x, edge_index, batch, edge_weight, n_graphs)
    nc = _build_program(meta)
    in_maps = _make_in_maps(meta, arrays, W1, b1, W2, b2, W3, b3, fcW, fcb)
    res = bass_utils.run_bass_kernel_spmd(
        nc, in_maps, core_ids=list(range(N_CORES)), trace=trace)
    ng = meta['n_graphs_core']
    y = np.concatenate([res.results[c]["y"][:ng[c]] for c in range(N_CORES)], axis=0)
    return y.astype(np.float32), res


def kernel(x, edge_index, batch, edge_weight, W1, b1, W2, b2, W3, b3, fcW, fcb):
    y, _ = run(np.asarray(x), np.asarray(edge_index), np.asarray(batch),
               np.asarray(edge_weight), W1, b1, W2, b2, W3, b3, fcW, fcb,
               n_graphs=512, trace=False)
    return y
